# revision 1
# baseline (speedup 1.0000x reference)
"""Causal self-attention (B=2, T=2048, C=1024, H=16, D=64) on 8 TRN2 NeuronCores.

Sharding (Megatron-style, per the hint): data-parallel over the batch (B=2)
and tensor-parallel over heads (16 heads -> 4 groups of 4). Core c handles
batch b = c // 4 and head group g = c % 4:
  - qkv:    computes x[b] @ w_attn[:, cols-of-its-4-heads]  (column split)
  - attn:   full causal attention for its 4 heads
  - proj:   y_heads @ w_proj[rows-of-its-4-heads]           (row split)
The 4 partial proj outputs per batch are summed on the host (+ b_proj).

Device layout notes:
  - All matmuls run in bf16 (inputs pre-cast/pre-transposed on host), fp32
    PSUM accumulation.
  - Scores are computed transposed: S'[s, t] = (k_s . q_t)/8, so softmax sums
    over s (the partition dim) come for free out of the AV matmul by
    augmenting V with a ones column:  yT_aug = [V | 1]^T @ exp(S').
    Row 64 of yT_aug is the softmax denominator per t.
  - exp has no max-subtraction: logits are O(1) for this input distribution
    (|logit| < ~10), so fp32/bf16 exp is safe and the normalization cancels.
"""

import os
import sys

sys.path.insert(0, "/opt/trn_rl_repo")

import numpy as np
import ml_dtypes

BF16 = ml_dtypes.bfloat16

B, T, C, H, D = 2, 2048, 1024, 16, 64
NCORES = 8
HG = 4          # heads per core
DQ = HG * D     # 256 qkv cols per core
CCH = C // 128  # 8 contraction chunks
NT = T // 128   # 16 token chunks of 128
NJ = T // 512   # 4 token tiles of 512

_NC_CACHE = {}


def build_nc(mm_dtype_name="bfloat16", loop=0, phases=("qkv", "attn", "proj"),
             attn_mode="full"):
    """loop=0: straight-line (graded path). loop=K>0: wrap the body in a
    device-side For_i repeat-K loop (timing builds only). phases: subset for
    bisection timing builds."""
    import contextlib
    import concourse.bacc as bacc
    import concourse.tile as tile
    from concourse import mybir

    mm_dt = getattr(mybir.dt, mm_dtype_name)
    f32 = mybir.dt.float32

    nc = bacc.Bacc("TRN2", target_bir_lowering=False, debug=False,
                   num_devices=NCORES)

    xT = nc.dram_tensor("xT", [C, T], mm_dt, kind="ExternalInput")
    wq = nc.dram_tensor("wq", [C, DQ], mm_dt, kind="ExternalInput")
    wk = nc.dram_tensor("wk", [C, DQ], mm_dt, kind="ExternalInput")
    wv = nc.dram_tensor("wv", [C, DQ], mm_dt, kind="ExternalInput")
    wp = nc.dram_tensor("wp", [DQ, C], mm_dt, kind="ExternalInput")
    bqk = nc.dram_tensor("bqk", [2, 2, 128], f32, kind="ExternalInput")  # [q/k, chunk, col]
    bv = nc.dram_tensor("bv", [128, DQ], f32, kind="ExternalInput")      # replicated
    mask = nc.dram_tensor("mask", [128, 128 + 4 * 512], mm_dt, kind="ExternalInput")
    out = nc.dram_tensor("out", [T, C], f32, kind="ExternalOutput")

    with tile.TileContext(nc) as tc:
        with (
            tc.tile_pool(name="const", bufs=1) as const,
            tc.tile_pool(name="acts", bufs=1) as acts,
            tc.tile_pool(name="work", bufs=4) as work,
            tc.tile_pool(name="ostage", bufs=3) as ostage,
            tc.tile_pool(name="psum", bufs=1, space="PSUM") as psum,
            tc.tile_pool(name="psums", bufs=1, space="PSUM") as psums,
            tc.For_i(0, loop, 1,
                     hint_engines=(mybir.EngineType.PE,
                                   mybir.EngineType.Activation,
                                   mybir.EngineType.DVE,
                                   mybir.EngineType.SP,
                                   mybir.EngineType.Pool))
            if loop else contextlib.nullcontext(),
        ):
            # ---- constants / weights ----
            xT_sb = const.tile([128, CCH, T], mm_dt)
            nc.sync.dma_start(out=xT_sb, in_=xT.rearrange("(c p) t -> p c t", p=128))
            wq_sb = const.tile([128, CCH, DQ], mm_dt)
            nc.sync.dma_start(out=wq_sb, in_=wq.rearrange("(c p) m -> p c m", p=128))
            wk_sb = const.tile([128, CCH, DQ], mm_dt)
            nc.sync.dma_start(out=wk_sb, in_=wk.rearrange("(c p) m -> p c m", p=128))
            wv_sb = const.tile([128, CCH, DQ], mm_dt)
            nc.sync.dma_start(out=wv_sb, in_=wv.rearrange("(c p) m -> p c m", p=128))
            wp_sb = const.tile([128, 2, C], mm_dt)
            nc.sync.dma_start(out=wp_sb, in_=wp.rearrange("(k p) n -> p k n", p=128))
            bqk_sb = const.tile([128, 2, 2, 1], f32)  # [col, q/k, chunk, 1]
            nc.sync.dma_start(out=bqk_sb, in_=bqk.rearrange("a m p -> p a m")[:, :, :, None])
            bv_sb = const.tile([128, DQ], f32)
            nc.sync.dma_start(out=bv_sb, in_=bv[:, :])
            # mask holds [ident(128) | 4 x 512 wide additive diag-block masks]
            maskc_sb = const.tile([128, 128], mm_dt)
            nc.sync.dma_start(out=maskc_sb, in_=mask[:, 0:128])
            maskw_sb = const.tile([128, 4, 512], mm_dt)
            nc.sync.dma_start(out=maskw_sb,
                              in_=mask[:, 128:].rearrange("p (a n) -> p a n", a=4))

            # ---- activations ----
            qd_sb = acts.tile([128, 2, T], mm_dt)   # [dcol, chunk, t]
            kd_sb = acts.tile([128, 2, T], mm_dt)
            v_sb = acts.tile([128, NT, HG * 65], mm_dt)  # per s-chunk: 4x [V_h | 1]
            yt_sb = acts.tile([128, 2, T], mm_dt)

            # ones columns of v_sb (col 64 of each head slot)
            ones_view = v_sb.rearrange("p s (h e) -> p s h e", e=65)[:, :, :, 64:65]
            nc.vector.memset(ones_view, 1.0)

            # bisection timing builds: initialize tensors a skipped phase
            # would have produced
            if "qkv" not in phases:
                nc.vector.memset(qd_sb, 0.5)
                nc.vector.memset(kd_sb, 0.5)
                nc.vector.memset(v_sb, 0.5)
            if "attn" not in phases:
                nc.vector.memset(yt_sb, 0.5)

            # ---- phase 1: qkv projections ----
            # Qd/Kd in d-major [dcol, t]; out tile = W_chunk^T @ xT_chunk.
            # Emission order (m=0 Q, m=0 K, V, m=1 Q, m=1 K) lets heads 0/1
            # attention start while heads 2/3 qkv still runs.
            def qk_proj(dst, wsb, qki, m):
                for j in range(NJ):
                    ps = psum.tile([128, 512], f32, tag="y", bufs=2, name="ps_qk")
                    for c in range(CCH):
                        nc.tensor.matmul(
                            ps,
                            lhsT=wsb[:, c, 128 * m:128 * m + 128],
                            rhs=xT_sb[:, c, 512 * j:512 * j + 512],
                            start=(c == 0), stop=(c == CCH - 1),
                        )
                    nc.vector.tensor_scalar_add(
                        dst[:, m, 512 * j:512 * j + 512], ps,
                        bqk_sb[:, qki, m, :],
                    )

            def v_proj():
                # V in s-major [t, vcol]; out tile = xT_chunk(t)^T @ Wv_chunk
                for tt in range(NT):
                    ps = psum.tile([128, 512], f32, tag="y", bufs=2, name="ps_v")
                    for c in range(CCH):
                        nc.tensor.matmul(
                            ps[:, 0:DQ],
                            lhsT=xT_sb[:, c, 128 * tt:128 * tt + 128],
                            rhs=wv_sb[:, c, :],
                            start=(c == 0), stop=(c == CCH - 1),
                        )
                    nc.vector.tensor_tensor(
                        v_sb.rearrange("p s (h e) -> p s h e", e=65)[:, tt, :, 0:64],
                        ps[:, 0:DQ].rearrange("p (h d) -> p h d", d=64),
                        bv_sb.rearrange("p (h d) -> p h d", d=64),
                        mybir.AluOpType.add,
                    )

            if "qkv" in phases:
                qk_proj(qd_sb, wq_sb, 0, 0)
                qk_proj(kd_sb, wk_sb, 1, 0)
                v_proj()
                qk_proj(qd_sb, wq_sb, 0, 1)
                qk_proj(kd_sb, wk_sb, 1, 1)

            # ---- phase 2: attention, per head ----
            from concourse.bass_isa import ReduceOp  # noqa: F401  (unused, kept for clarity)

            # ---- phase 2+3: attention (j outer, h inner) + interleaved proj
            # Software-pipelined emission: AV of unit k is emitted after the
            # S' matmuls of unit k+2, so the in-order PE stream never blocks
            # on the ~1.2us ACT exp latency.
            exp_f = mybir.ActivationFunctionType.Exp
            LAG = int(os.environ.get("ATTN_LAG", "2"))

            pending = []  # queue of emitted-S'/exp units awaiting AV emission

            def flush_unit():
                u = pending.pop(0)
                for mmargs in u["av"]:
                    nc.tensor.matmul(**mmargs)
                if u["fin"] is not None:
                    h, j, yps = u["fin"]
                    m, roff = divmod(h, 2)
                    roff *= 64
                    r = work.tile([1, 512], f32, tag="r", bufs=2, name="r")
                    nc.vector.reciprocal(r, yps[64:65, :])
                    rr = work.tile([64, 512], f32, tag="rr", bufs=2, name="rr")
                    nc.gpsimd.partition_broadcast(rr, r)
                    nc.vector.tensor_tensor(
                        yt_sb[roff:roff + 64, m, 512 * j:512 * j + 512],
                        yps[0:64, :], rr, mybir.AluOpType.mult,
                    )

            def attn_head_window(h, j):
                m, roff = divmod(h, 2)
                roff *= 64
                kd_h = kd_sb[roff:roff + 64, m, :]
                qd_h = qd_sb[roff:roff + 64, m, :]
                jwin = slice(512 * j, 512 * (j + 1))
                yps = None
                if attn_mode != "noav":
                    yps = psum.tile([128, 512], f32, tag="y", bufs=2, name="yps")
                nI = 4 * j + 4
                # units of 2 s-chunks -> one [128,1024] exp. Diagonal-block
                # chunks (i >= 4j) are computed full-width with the additive
                # causal mask folded into the PE accumulation group.
                for i2 in range(2 * j + 2):
                    sps = psums.tile([128, 1024], f32, tag="s", bufs=3, name="sps")
                    nomask = "nomask" in attn_mode
                    for u in (0, 1):
                        i = 2 * i2 + u
                        d = i - 4 * j  # >= 0 for diagonal-block chunks
                        nc.tensor.matmul(
                            sps[:, 512 * u:512 * u + 512],
                            lhsT=kd_h[:, 128 * i:128 * i + 128],
                            rhs=qd_h[:, jwin],
                            start=True, stop=(d < 0 or nomask),
                        )
                        if d >= 0 and not nomask:
                            nc.tensor.matmul(
                                sps[:, 512 * u:512 * u + 128 * (d + 1)],
                                lhsT=maskc_sb,                   # identity
                                rhs=maskw_sb[:, d, 0:128 * (d + 1)],
                                start=False, stop=True,
                            )
                    pt = work.tile([128, 1024], mm_dt, tag="p", bufs=6, name="pt")
                    f = exp_f if "expcopy" not in attn_mode else \
                        mybir.ActivationFunctionType.Copy
                    nc.scalar.activation(pt, sps, f, scale=0.125)
                    if attn_mode == "noav":
                        continue
                    av = [dict(out=yps[0:65, :],
                               lhsT=v_sb[:, 2 * i2 + u, 65 * h:65 * h + 65],
                               rhs=pt[:, 512 * u:512 * u + 512],
                               start=(2 * i2 + u == 0),
                               stop=(2 * i2 + u == nI - 1))
                          for u in (0, 1)]
                    pending.append(dict(
                        av=av, fin=(h, j, yps) if i2 == 2 * j + 1 else None))
                    while len(pending) > LAG:
                        flush_unit()

            def proj_window(j):
                for tt in range(4 * j, 4 * j + 4):
                    os_sb = ostage.tile([128, C], f32, tag="osb", name="os_sb")
                    for n2 in range(2):
                        ps = psum.tile([128, 512], f32, tag="y", bufs=2,
                                       name="ps_o")
                        for kc in range(2):
                            nc.tensor.matmul(
                                ps,
                                lhsT=yt_sb[:, kc, 128 * tt:128 * tt + 128],
                                rhs=wp_sb[:, kc, 512 * n2:512 * n2 + 512],
                                start=(kc == 0), stop=(kc == 1),
                            )
                        nc.vector.tensor_copy(os_sb[:, 512 * n2:512 * n2 + 512],
                                              ps)
                    nc.sync.dma_start(out=out[128 * tt:128 * tt + 128, :],
                                      in_=os_sb)

            hlist = [0, 2, 0, 2] if "evenheads" in attn_mode else range(HG)
            if "attn" in phases:
                for j in range(NJ):
                    for h in hlist:
                        attn_head_window(h, j)
                    if "proj" in phases:
                        while pending:  # finish j's AVs before proj reads yt
                            flush_unit()
                        proj_window(j)
                while pending:
                    flush_unit()
            elif "proj" in phases:
                for j in range(NJ):
                    proj_window(j)

    nc.finalize()
    return nc


def make_in_maps(x, w_attn, b_attn, w_proj):
    x = np.asarray(x, dtype=np.float32)
    w_attn = np.asarray(w_attn, dtype=np.float32)
    b_attn = np.asarray(b_attn, dtype=np.float32)
    w_proj = np.asarray(w_proj, dtype=np.float32)

    ident = np.eye(128, dtype=np.float32)
    tri = np.where(np.triu(np.ones((128, 128), bool)), 0.0, -3000.0)
    blocks = [ident]
    for d in range(4):
        blk = np.zeros((128, 512), np.float32)
        blk[:, :128 * d] = -3000.0
        blk[:, 128 * d:128 * d + 128] = tri
        blocks.append(blk)
    mask_np = np.concatenate(blocks, axis=1).astype(BF16)  # [128, 128+2048]
    in_maps = []
    for core in range(NCORES):
        b, g = divmod(core, 4)
        cq = slice(0 * C + g * DQ, 0 * C + (g + 1) * DQ)
        ck = slice(1 * C + g * DQ, 1 * C + (g + 1) * DQ)
        cv = slice(2 * C + g * DQ, 2 * C + (g + 1) * DQ)
        bq = b_attn[cq]
        bk = b_attn[ck]
        bqk = np.stack([bq.reshape(2, 128), bk.reshape(2, 128)]).astype(np.float32)
        bv = np.broadcast_to(b_attn[cv], (128, DQ)).copy().astype(np.float32)
        in_maps.append({
            "xT": np.ascontiguousarray(x[b].T).astype(BF16),
            "wq": np.ascontiguousarray(w_attn[:, cq]).astype(BF16),
            "wk": np.ascontiguousarray(w_attn[:, ck]).astype(BF16),
            "wv": np.ascontiguousarray(w_attn[:, cv]).astype(BF16),
            "wp": np.ascontiguousarray(w_proj[g * DQ:(g + 1) * DQ, :]).astype(BF16),
            "bqk": bqk,
            "bv": bv,
            "mask": mask_np,
        })
    return in_maps


def kernel(x, w_attn, b_attn, w_proj, b_proj):
    from concourse.bass_utils import run_bass_kernel_spmd

    if "nc" not in _NC_CACHE:
        _NC_CACHE["nc"] = build_nc()
    nc = _NC_CACHE["nc"]

    in_maps = make_in_maps(x, w_attn, b_attn, w_proj)
    res = run_bass_kernel_spmd(nc, in_maps, list(range(NCORES)))

    b_proj = np.asarray(b_proj, dtype=np.float32)
    out = np.zeros((B, T, C), np.float32)
    for core in range(NCORES):
        b = core // 4
        out[b] += res.results[core]["out"]
    out += b_proj[None, None, :]
    return out



# revision 9
# speedup vs baseline: 203.1904x; 203.1904x over previous
"""Causal self-attention (B=2, T=2048, C=1024, H=16, D=64) on 8 TRN2 NeuronCores.

Sharding (Megatron-style, per the hint): data-parallel over the batch (B=2)
and tensor-parallel over heads (16 heads -> 4 groups of 4). Core c handles
batch b = c // 4 and head group g = c % 4:
  - qkv:    computes x[b] @ w_attn[:, cols-of-its-4-heads]  (column split)
  - attn:   full causal attention for its 4 heads
  - proj:   y_heads @ w_proj[rows-of-its-4-heads]           (row split)
The 4 partial proj outputs per batch are summed on the host (+ b_proj).

Device layout notes:
  - All matmuls run in bf16 (inputs pre-cast/pre-transposed on host), fp32
    PSUM accumulation.
  - Scores are computed transposed: S'[s, t] = (k_s . q_t)/8, so softmax sums
    over s (the partition dim) come for free out of the AV matmul by
    augmenting V with a ones column:  yT_aug = [V | 1]^T @ exp(S').
    Row 64 of yT_aug is the softmax denominator per t.
  - exp has no max-subtraction: logits are O(1) for this input distribution
    (|logit| < ~10), so fp32/bf16 exp is safe and the normalization cancels.
  - Diagonal-window S'/mask/AV matmuls are narrowed to skip fully-masked
    column ranges (exp still runs full-width; the stale columns are never
    read by the narrowed AV).
  - Input DMAs are issued on the ACT queue (SP carries the output DMAs), so
    next-iteration input prefetch does not serialize behind output drain.
  - proj runs one q-window behind attention (proj(j-1) between head 1 and
    head 2 of window j) so the PE never waits for the softmax-normalize
    chain; proj PSUM lives in the "s" ring and its PSUM->SBUF copies run on
    the Pool engine, keeping DVE free for the normalize chain.
  - Partial proj outputs are DMA'd out in bf16 (summed in fp32 on host).
"""

import os
import sys

sys.path.insert(0, "/opt/trn_rl_repo")

import numpy as np
import ml_dtypes

BF16 = ml_dtypes.bfloat16

B, T, C, H, D = 2, 2048, 1024, 16, 64
NCORES = 8
HG = 4          # heads per core
DQ = HG * D     # 256 qkv cols per core
CCH = C // 128  # 8 contraction chunks
NT = T // 128   # 16 token chunks of 128
NJ = T // 512   # 4 token tiles of 512

_NC_CACHE = {}


def build_nc(mm_dtype_name="bfloat16", loop=0, phases=("qkv", "attn", "proj"),
             attn_mode="full", dma_eng="act", copy_eng="dve"):
    """loop=0: straight-line (graded path). loop=K>0: wrap the body in a
    device-side For_i repeat-K loop (timing builds only). phases: subset for
    bisection timing builds."""
    import contextlib
    import concourse.bacc as bacc
    import concourse.tile as tile
    from concourse import mybir

    mm_dt = getattr(mybir.dt, mm_dtype_name)
    f32 = mybir.dt.float32

    nc = bacc.Bacc("TRN2", target_bir_lowering=False, debug=False,
                   num_devices=NCORES)

    xT = nc.dram_tensor("xT", [C, T], mm_dt, kind="ExternalInput")
    wq = nc.dram_tensor("wq", [C, DQ], mm_dt, kind="ExternalInput")
    wk = nc.dram_tensor("wk", [C, DQ], mm_dt, kind="ExternalInput")
    wv = nc.dram_tensor("wv", [C, DQ], mm_dt, kind="ExternalInput")
    wp = nc.dram_tensor("wp", [DQ, C], mm_dt, kind="ExternalInput")
    bqk = nc.dram_tensor("bqk", [2, 2, 128], f32, kind="ExternalInput")  # [q/k, chunk, col]
    bv = nc.dram_tensor("bv", [128, DQ], f32, kind="ExternalInput")      # replicated
    mask = nc.dram_tensor("mask", [128, 128 + 4 * 512], mm_dt, kind="ExternalInput")
    out = nc.dram_tensor("out", [T, C], mm_dt, kind="ExternalOutput")

    with tile.TileContext(nc) as tc:
        with (
            tc.tile_pool(name="const", bufs=1) as const,
            tc.tile_pool(name="acts", bufs=1) as acts,
            tc.tile_pool(name="work", bufs=4) as work,
            tc.tile_pool(name="ostage", bufs=3) as ostage,
            tc.tile_pool(name="psum", bufs=1, space="PSUM") as psum,
            tc.tile_pool(name="psums", bufs=1, space="PSUM") as psums,
            tc.For_i(0, loop, 1,
                     hint_engines=(mybir.EngineType.PE,
                                   mybir.EngineType.Activation,
                                   mybir.EngineType.DVE,
                                   mybir.EngineType.SP,
                                   mybir.EngineType.Pool))
            if loop else contextlib.nullcontext(),
        ):
            # ---- constants / weights (issued on the ACT DMA queue, ordered
            # so qkv compute can start as soon as its operands land) ----
            wq_sb = const.tile([128, CCH, DQ], mm_dt)
            xT_sb = const.tile([128, CCH, T], mm_dt)
            wk_sb = const.tile([128, CCH, DQ], mm_dt)
            wv_sb = const.tile([128, CCH, DQ], mm_dt)
            wp_sb = const.tile([128, 2, C], mm_dt)
            bqk_sb = const.tile([128, 2, 2, 1], f32)  # [col, q/k, chunk, 1]
            bv_sb = const.tile([128, DQ], f32)
            # mask holds [ident(128) | 4 x 512 wide additive diag-block masks]
            maskc_sb = const.tile([128, 128], mm_dt)
            maskw_sb = const.tile([128, 4, 512], mm_dt)

            xT_r = xT.rearrange("(c p) t -> p c t", p=128)
            ldq = nc.scalar if dma_eng == "act" else nc.sync

            def xpiece(p):
                tw = slice(512 * p, 512 * p + 512)
                ldq.dma_start(out=xT_sb[:, :, tw], in_=xT_r[:, :, tw])

            ldq.dma_start(out=wq_sb, in_=wq.rearrange("(c p) m -> p c m", p=128))
            xpiece(0)
            ldq.dma_start(out=wk_sb, in_=wk.rearrange("(c p) m -> p c m", p=128))
            ldq.dma_start(out=bqk_sb, in_=bqk.rearrange("a m p -> p a m")[:, :, :, None])
            xpiece(1)
            ldq.dma_start(out=wv_sb, in_=wv.rearrange("(c p) m -> p c m", p=128))
            ldq.dma_start(out=bv_sb, in_=bv[:, :])
            xpiece(2)
            ldq.dma_start(out=wp_sb, in_=wp.rearrange("(k p) n -> p k n", p=128))
            ldq.dma_start(out=maskc_sb, in_=mask[:, 0:128])
            ldq.dma_start(out=maskw_sb,
                          in_=mask[:, 128:].rearrange("p (a n) -> p a n", a=4))
            xpiece(3)

            # ---- activations ----
            qd_sb = acts.tile([128, 2, T], mm_dt)   # [dcol, chunk, t]
            kd_sb = acts.tile([128, 2, T], mm_dt)
            v_sb = acts.tile([128, NT, HG * 65], mm_dt)  # per s-chunk: 4x [V_h | 1]
            yt_sb = acts.tile([128, 2, T], mm_dt)

            # ones columns of v_sb (col 64 of each head slot)
            ones_view = v_sb.rearrange("p s (h e) -> p s h e", e=65)[:, :, :, 64:65]
            nc.vector.memset(ones_view, 1.0)

            # bisection timing builds: initialize tensors a skipped phase
            # would have produced
            if "qkv" not in phases:
                nc.vector.memset(qd_sb, 0.5)
                nc.vector.memset(kd_sb, 0.5)
                nc.vector.memset(v_sb, 0.5)
            if "attn" not in phases:
                nc.vector.memset(yt_sb, 0.5)

            # ---- phase 1: qkv projections ----
            # Qd/Kd in d-major [dcol, t]; out tile = W_chunk^T @ xT_chunk.
            # Emission order (m=0 Q, m=0 K, V, m=1 Q, m=1 K) lets heads 0/1
            # attention start while heads 2/3 qkv still runs.
            def qk_proj_j(dst, wsb, qki, m, j):
                ps = psum.tile([128, 512], f32, tag="y", bufs=2, name="ps_qk")
                for c in range(CCH):
                    nc.tensor.matmul(
                        ps,
                        lhsT=wsb[:, c, 128 * m:128 * m + 128],
                        rhs=xT_sb[:, c, 512 * j:512 * j + 512],
                        start=(c == 0), stop=(c == CCH - 1),
                    )
                nc.vector.tensor_scalar_add(
                    dst[:, m, 512 * j:512 * j + 512], ps,
                    bqk_sb[:, qki, m, :],
                )

            def v_proj_tt(tt):
                # V in s-major [t, vcol]; out tile = xT_chunk(t)^T @ Wv_chunk
                ps = psum.tile([128, 512], f32, tag="y", bufs=2, name="ps_v")
                for c in range(CCH):
                    nc.tensor.matmul(
                        ps[:, 0:DQ],
                        lhsT=xT_sb[:, c, 128 * tt:128 * tt + 128],
                        rhs=wv_sb[:, c, :],
                        start=(c == 0), stop=(c == CCH - 1),
                    )
                nc.vector.tensor_tensor(
                    v_sb.rearrange("p s (h e) -> p s h e", e=65)[:, tt, :, 0:64],
                    ps[:, 0:DQ].rearrange("p (h d) -> p h d", d=64),
                    bv_sb.rearrange("p (h d) -> p h d", d=64),
                    mybir.AluOpType.add,
                )

            if "qkv" in phases:
                # piece-interleaved: q/k/v for xT piece p emitted together so
                # PE work rate-matches the xT piece DMAs at iteration start
                for j in range(NJ):
                    qk_proj_j(qd_sb, wq_sb, 0, 0, j)
                    qk_proj_j(kd_sb, wk_sb, 1, 0, j)
                    for tt in range(4 * j, 4 * j + 4):
                        v_proj_tt(tt)
                for j in range(NJ):
                    qk_proj_j(qd_sb, wq_sb, 0, 1, j)
                    qk_proj_j(kd_sb, wk_sb, 1, 1, j)

            # ---- phase 2+3: attention (j outer, h inner) with proj lagging
            # one window behind (proj(j-1) emitted between head 1 and head 2
            # of window j). Software-pipelined AV emission: AV of unit k is
            # emitted after the S' matmuls of unit k+LAG, so the in-order PE
            # stream never blocks on the ~1.2us ACT exp latency.
            exp_f = mybir.ActivationFunctionType.Exp
            LAG = int(os.environ.get("ATTN_LAG", "2"))

            pending = []  # queue of emitted-S'/exp units awaiting AV emission

            def flush_unit():
                u = pending.pop(0)
                for mmargs in u["av"]:
                    nc.tensor.matmul(**mmargs)
                if u["fin"] is not None:
                    h, j, yps = u["fin"]
                    m, roff = divmod(h, 2)
                    roff *= 64
                    r = work.tile([1, 512], f32, tag="r", bufs=2, name="r")
                    nc.vector.reciprocal(r, yps[64:65, :])
                    rr = work.tile([64, 512], f32, tag="rr", bufs=2, name="rr")
                    nc.gpsimd.partition_broadcast(rr, r)
                    nc.vector.tensor_tensor(
                        yt_sb[roff:roff + 64, m, 512 * j:512 * j + 512],
                        yps[0:64, :], rr, mybir.AluOpType.mult,
                    )

            def attn_head_window(h, j):
                m, roff = divmod(h, 2)
                roff *= 64
                kd_h = kd_sb[roff:roff + 64, m, :]
                qd_h = qd_sb[roff:roff + 64, m, :]
                jwin = slice(512 * j, 512 * (j + 1))
                yps = None
                if attn_mode != "noav":
                    yps = psum.tile([128, 512], f32, tag="y", bufs=2, name="yps")
                nI = 4 * j + 4
                # units of 2 s-chunks -> one [128,1024] exp. Diagonal-block
                # chunks (d = i - 4j >= 0) are narrowed: only columns
                # >= 128*d of the 512-wide q-window are computed (the rest
                # are fully causally masked); the 128-wide triangular mask
                # block is folded into the PE accumulation group.
                for i2 in range(2 * j + 2):
                    sps = psums.tile([128, 1024], f32, tag="s", bufs=3, name="sps")
                    nomask = "nomask" in attn_mode
                    for u in (0, 1):
                        i = 2 * i2 + u
                        d = i - 4 * j  # >= 0 for diagonal-block chunks
                        off = 128 * d if (d > 0 and not nomask) else 0
                        nc.tensor.matmul(
                            sps[:, 512 * u + off:512 * u + 512],
                            lhsT=kd_h[:, 128 * i:128 * i + 128],
                            rhs=qd_h[:, 512 * j + off:512 * j + 512],
                            start=True, stop=(d < 0 or nomask),
                        )
                        if d >= 0 and not nomask:
                            nc.tensor.matmul(
                                sps[:, 512 * u + off:512 * u + off + 128],
                                lhsT=maskc_sb,                   # identity
                                rhs=maskw_sb[:, d, off:off + 128],
                                start=False, stop=True,
                            )
                    pt = work.tile([128, 1024], mm_dt, tag="p", bufs=6, name="pt")
                    f = exp_f if "expcopy" not in attn_mode else \
                        mybir.ActivationFunctionType.Copy
                    offs = []
                    for u in (0, 1):
                        d = 2 * i2 + u - 4 * j
                        offs.append(128 * d if (d > 0 and not nomask) else 0)
                    if offs == [0, 0]:
                        nc.scalar.activation(pt, sps, f, scale=0.125)
                    else:
                        for u in (0, 1):
                            sl = slice(512 * u + offs[u], 512 * u + 512)
                            nc.scalar.activation(pt[:, sl], sps[:, sl], f,
                                                 scale=0.125)
                    if attn_mode == "noav":
                        continue
                    av = []
                    for u in (0, 1):
                        i = 2 * i2 + u
                        d = i - 4 * j
                        off = 128 * d if d > 0 else 0
                        av.append(dict(
                            out=yps[0:65, off:512],
                            lhsT=v_sb[:, i, 65 * h:65 * h + 65],
                            rhs=pt[:, 512 * u + off:512 * u + 512],
                            start=(i == 0),
                            stop=(i == nI - 1)))
                    pending.append(dict(
                        av=av, fin=(h, j, yps) if i2 == 2 * j + 1 else None))
                    while len(pending) > LAG:
                        flush_unit()

            def proj_window(j):
                for tt in range(4 * j, 4 * j + 4):
                    pso = psums.tile([128, 1024], f32, tag="s", bufs=3,
                                     name="pso")
                    for n2 in range(2):
                        for kc in range(2):
                            nc.tensor.matmul(
                                pso[:, 512 * n2:512 * n2 + 512],
                                lhsT=yt_sb[:, kc, 128 * tt:128 * tt + 128],
                                rhs=wp_sb[:, kc, 512 * n2:512 * n2 + 512],
                                start=(kc == 0), stop=(kc == 1),
                            )
                    os_sb = ostage.tile([128, C], mm_dt, tag="osb", name="os_sb")
                    ceng = nc.gpsimd if copy_eng == "pool" else nc.vector
                    ceng.tensor_copy(os_sb, pso)
                    nc.sync.dma_start(out=out[128 * tt:128 * tt + 128, :],
                                      in_=os_sb)

            hlist = [0, 2, 0, 2] if "evenheads" in attn_mode else list(range(HG))
            if "attn" in phases:
                for j in range(NJ):
                    for hx, h in enumerate(hlist):
                        attn_head_window(h, j)
                        if hx == 1 and j > 0 and "proj" in phases:
                            proj_window(j - 1)
                while pending:
                    flush_unit()
                if "proj" in phases:
                    proj_window(NJ - 1)
            elif "proj" in phases:
                for j in range(NJ):
                    proj_window(j)

    nc.finalize()
    return nc


def make_in_maps(x, w_attn, b_attn, w_proj):
    x = np.asarray(x, dtype=np.float32)
    w_attn = np.asarray(w_attn, dtype=np.float32)
    b_attn = np.asarray(b_attn, dtype=np.float32)
    w_proj = np.asarray(w_proj, dtype=np.float32)

    ident = np.eye(128, dtype=np.float32)
    tri = np.where(np.triu(np.ones((128, 128), bool)), 0.0, -3000.0)
    blocks = [ident]
    for d in range(4):
        blk = np.zeros((128, 512), np.float32)
        blk[:, :128 * d] = -3000.0
        blk[:, 128 * d:128 * d + 128] = tri
        blocks.append(blk)
    mask_np = np.concatenate(blocks, axis=1).astype(BF16)  # [128, 128+2048]
    in_maps = []
    for core in range(NCORES):
        b, g = divmod(core, 4)
        cq = slice(0 * C + g * DQ, 0 * C + (g + 1) * DQ)
        ck = slice(1 * C + g * DQ, 1 * C + (g + 1) * DQ)
        cv = slice(2 * C + g * DQ, 2 * C + (g + 1) * DQ)
        bq = b_attn[cq]
        bk = b_attn[ck]
        bqk = np.stack([bq.reshape(2, 128), bk.reshape(2, 128)]).astype(np.float32)
        bv = np.broadcast_to(b_attn[cv], (128, DQ)).copy().astype(np.float32)
        in_maps.append({
            "xT": np.ascontiguousarray(x[b].T).astype(BF16),
            "wq": np.ascontiguousarray(w_attn[:, cq]).astype(BF16),
            "wk": np.ascontiguousarray(w_attn[:, ck]).astype(BF16),
            "wv": np.ascontiguousarray(w_attn[:, cv]).astype(BF16),
            "wp": np.ascontiguousarray(w_proj[g * DQ:(g + 1) * DQ, :]).astype(BF16),
            "bqk": bqk,
            "bv": bv,
            "mask": mask_np,
        })
    return in_maps


def kernel(x, w_attn, b_attn, w_proj, b_proj):
    from concourse.bass_utils import run_bass_kernel_spmd

    if "nc" not in _NC_CACHE:
        _NC_CACHE["nc"] = build_nc()
    nc = _NC_CACHE["nc"]

    in_maps = make_in_maps(x, w_attn, b_attn, w_proj)
    res = run_bass_kernel_spmd(nc, in_maps, list(range(NCORES)))

    b_proj = np.asarray(b_proj, dtype=np.float32)
    out = np.zeros((B, T, C), np.float32)
    for core in range(NCORES):
        b = core // 4
        out[b] += res.results[core]["out"].astype(np.float32)
    out += b_proj[None, None, :]
    return out


# revision 26
# speedup vs baseline: 333.9801x; 1.6437x over previous
"""Causal self-attention (B=2, T=2048, C=1024, H=16, D=64) on 8 TRN2 NeuronCores.

Sharding (Megatron-style, per the hint): data-parallel over the batch (B=2)
and tensor-parallel over heads (16 heads -> 4 groups of 4). Core c handles
batch b = c // 4 and head group g = c % 4:
  - qkv:    computes x[b] @ w_attn[:, cols-of-its-4-heads]  (column split)
  - attn:   full causal attention for its 4 heads
  - proj:   y_heads @ w_proj[rows-of-its-4-heads]           (row split)
The 4 partial proj outputs per batch are summed on the host (+ b_proj).

Device layout notes:
  - All matmuls run in bf16 (inputs pre-cast/pre-transposed on host), fp32
    PSUM accumulation.
  - Scores are computed transposed: S'[s, t] = (k_s . q_t)/8, so softmax sums
    over s (the partition dim) come for free out of the AV matmul by
    augmenting V with a ones column:  yT_aug = [V | 1]^T @ exp(S').
    Row 64 of yT_aug is the softmax denominator per t.
  - exp has no max-subtraction: logits are O(1) for this input distribution
    (|logit| < ~10), so fp32/bf16 exp is safe and the normalization cancels.
  - Diagonal-window S'/mask/AV matmuls are narrowed to skip fully-masked
    column ranges (exp still runs full-width; the stale columns are never
    read by the narrowed AV).
  - Input DMAs are issued on the ACT queue (SP carries the output DMAs), so
    next-iteration input prefetch does not serialize behind output drain.
  - proj runs one q-window behind attention (proj(j-1) between head 1 and
    head 2 of window j) so the PE never waits for the softmax-normalize
    chain; proj PSUM lives in the "s" ring and its PSUM->SBUF copies run on
    the Pool engine, keeping DVE free for the normalize chain.
  - Partial proj outputs are DMA'd out in bf16 (summed in fp32 on host).
"""

import os
import sys

sys.path.insert(0, "/opt/trn_rl_repo")

import numpy as np
import ml_dtypes

BF16 = ml_dtypes.bfloat16

B, T, C, H, D = 2, 2048, 1024, 16, 64
NCORES = 8
HG = 4          # heads per core
DQ = HG * D     # 256 qkv cols per core
CCH = C // 128  # 8 contraction chunks
NT = T // 128   # 16 token chunks of 128
NJ = T // 512   # 4 token tiles of 512

_NC_CACHE = {}


def build_nc(mm_dtype_name="bfloat16", loop=0, phases=("qkv", "attn", "proj"),
             attn_mode="full", dma_eng="act", copy_eng="dve",
             narrow=True, interleave=True, out_bf16=True, dvemask=True):
    """loop=0: straight-line (graded path). loop=K>0: wrap the body in a
    device-side For_i repeat-K loop (timing builds only). phases: subset for
    bisection timing builds."""
    import contextlib
    import concourse.bacc as bacc
    import concourse.tile as tile
    from concourse import mybir

    mm_dt = getattr(mybir.dt, mm_dtype_name)
    f32 = mybir.dt.float32
    assert narrow or not dvemask, "dvemask requires narrow"

    nc = bacc.Bacc("TRN2", target_bir_lowering=False, debug=False,
                   num_devices=NCORES)

    xT = nc.dram_tensor("xT", [C, T], mm_dt, kind="ExternalInput")
    wq = nc.dram_tensor("wq", [C, DQ], mm_dt, kind="ExternalInput")
    wk = nc.dram_tensor("wk", [C, DQ], mm_dt, kind="ExternalInput")
    wv = nc.dram_tensor("wv", [C, DQ], mm_dt, kind="ExternalInput")
    wp = nc.dram_tensor("wp", [DQ, C], mm_dt, kind="ExternalInput")
    bqk = nc.dram_tensor("bqk", [2, 2, 128], f32, kind="ExternalInput")  # [q/k, chunk, col]
    bv = nc.dram_tensor("bv", [128, DQ], f32, kind="ExternalInput")      # replicated
    mask = nc.dram_tensor("mask", [128, 128 + 4 * 512 + 128], mm_dt,
                          kind="ExternalInput")
    out_dt = mm_dt if out_bf16 else f32
    out = nc.dram_tensor("out", [T, C], out_dt, kind="ExternalOutput")

    with tile.TileContext(nc) as tc:
        with (
            tc.tile_pool(name="const", bufs=1) as const,
            tc.tile_pool(name="acts", bufs=1) as acts,
            tc.tile_pool(name="work", bufs=4) as work,
            tc.tile_pool(name="ostage", bufs=3) as ostage,
            tc.tile_pool(name="psum", bufs=1, space="PSUM") as psum,
            tc.tile_pool(name="psums", bufs=1, space="PSUM") as psums,
            tc.For_i(0, loop, 1,
                     hint_engines=(mybir.EngineType.PE,
                                   mybir.EngineType.Activation,
                                   mybir.EngineType.DVE,
                                   mybir.EngineType.SP,
                                   mybir.EngineType.Pool))
            if loop else contextlib.nullcontext(),
        ):
            # ---- constants / weights (issued on the ACT DMA queue, ordered
            # so qkv compute can start as soon as its operands land) ----
            wq_sb = const.tile([128, CCH, DQ], mm_dt)
            xT_sb = const.tile([128, CCH, T], mm_dt)
            wk_sb = const.tile([128, CCH, DQ], mm_dt)
            wv_sb = const.tile([128, CCH, DQ], mm_dt)
            wp_sb = const.tile([128, 2, C], mm_dt)
            bqk_sb = const.tile([128, 2, 2, 1], f32)  # [col, q/k, chunk, 1]
            bv_sb = const.tile([128, DQ], f32)
            # mask holds [ident(128) | 4 x 512 additive diag masks | 0/1 tri]
            maskc_sb = const.tile([128, 128], mm_dt)
            maskw_sb = const.tile([128, 4, 512], mm_dt)
            maskt_sb = const.tile([128, 128], mm_dt)

            xT_r = xT.rearrange("(c p) t -> p c t", p=128)
            # xT pieces on the ACT queue, everything else on SP (in parallel;
            # SP's out-DMAs only queue up later in the body).
            ldq = nc.scalar if dma_eng == "act" else nc.sync
            ldw = nc.sync if dma_eng == "act" else nc.scalar

            def xpiece(p):
                tw = slice(512 * p, 512 * p + 512)
                ldq.dma_start(out=xT_sb[:, :, tw], in_=xT_r[:, :, tw])

            ldw.dma_start(out=wq_sb, in_=wq.rearrange("(c p) m -> p c m", p=128))
            xpiece(0)
            ldw.dma_start(out=wk_sb, in_=wk.rearrange("(c p) m -> p c m", p=128))
            ldw.dma_start(out=bqk_sb, in_=bqk.rearrange("a m p -> p a m")[:, :, :, None])
            xpiece(1)
            ldw.dma_start(out=wv_sb, in_=wv.rearrange("(c p) m -> p c m", p=128))
            ldw.dma_start(out=bv_sb, in_=bv[:, :])
            xpiece(2)
            ldw.dma_start(out=wp_sb, in_=wp.rearrange("(k p) n -> p k n", p=128))
            ldw.dma_start(out=maskc_sb, in_=mask[:, 0:128])
            ldw.dma_start(out=maskw_sb,
                          in_=mask[:, 128:128 + 2048].rearrange(
                              "p (a n) -> p a n", a=4))
            ldw.dma_start(out=maskt_sb, in_=mask[:, 128 + 2048:])
            xpiece(3)

            # ---- activations ----
            qd_sb = acts.tile([128, 2, T], mm_dt)   # [dcol, chunk, t]
            kd_sb = acts.tile([128, 2, T], mm_dt)
            v_sb = acts.tile([128, NT, HG * 65], mm_dt)  # per s-chunk: 4x [V_h | 1]
            yt_sb = acts.tile([128, 2, T], mm_dt)

            # ones columns of v_sb (col 64 of each head slot)
            ones_view = v_sb.rearrange("p s (h e) -> p s h e", e=65)[:, :, :, 64:65]
            nc.vector.memset(ones_view, 1.0)

            # bisection timing builds: initialize tensors a skipped phase
            # would have produced
            if "qkv" not in phases:
                nc.vector.memset(qd_sb, 0.5)
                nc.vector.memset(kd_sb, 0.5)
                nc.vector.memset(v_sb, 0.5)
            if "attn" not in phases:
                nc.vector.memset(yt_sb, 0.5)

            # ---- phase 1: qkv projections ----
            # Qd/Kd in d-major [dcol, t]; out tile = W_chunk^T @ xT_chunk.
            # Emission order (m=0 Q, m=0 K, V, m=1 Q, m=1 K) lets heads 0/1
            # attention start while heads 2/3 qkv still runs.
            def qk_proj_j(dst, wsb, qki, m, j):
                ps = psum.tile([128, 512], f32, tag="y", bufs=2, name="ps_qk")
                for c in range(CCH):
                    nc.tensor.matmul(
                        ps,
                        lhsT=wsb[:, c, 128 * m:128 * m + 128],
                        rhs=xT_sb[:, c, 512 * j:512 * j + 512],
                        start=(c == 0), stop=(c == CCH - 1),
                    )
                nc.vector.tensor_scalar_add(
                    dst[:, m, 512 * j:512 * j + 512], ps,
                    bqk_sb[:, qki, m, :],
                )

            def v_proj_tt(tt):
                # V in s-major [t, vcol]; out tile = xT_chunk(t)^T @ Wv_chunk
                ps = psum.tile([128, 512], f32, tag="y", bufs=2, name="ps_v")
                for c in range(CCH):
                    nc.tensor.matmul(
                        ps[:, 0:DQ],
                        lhsT=xT_sb[:, c, 128 * tt:128 * tt + 128],
                        rhs=wv_sb[:, c, :],
                        start=(c == 0), stop=(c == CCH - 1),
                    )
                nc.vector.tensor_tensor(
                    v_sb.rearrange("p s (h e) -> p s h e", e=65)[:, tt, :, 0:64],
                    ps[:, 0:DQ].rearrange("p (h d) -> p h d", d=64),
                    bv_sb.rearrange("p (h d) -> p h d", d=64),
                    mybir.AluOpType.add,
                )

            if "qkv" in phases:
                # piece-interleaved: q/k/v for xT piece p emitted together so
                # PE work rate-matches the xT piece DMAs at iteration start
                for j in range(NJ):
                    qk_proj_j(qd_sb, wq_sb, 0, 0, j)
                    qk_proj_j(kd_sb, wk_sb, 1, 0, j)
                    for tt in range(4 * j, 4 * j + 4):
                        v_proj_tt(tt)
                for j in range(NJ):
                    qk_proj_j(qd_sb, wq_sb, 0, 1, j)
                    qk_proj_j(kd_sb, wk_sb, 1, 1, j)

            # ---- phase 2+3: attention (j outer, h inner) with proj lagging
            # one window behind (proj(j-1) emitted between head 1 and head 2
            # of window j). Software-pipelined AV emission: AV of unit k is
            # emitted after the S' matmuls of unit k+LAG, so the in-order PE
            # stream never blocks on the ~1.2us ACT exp latency.
            exp_f = mybir.ActivationFunctionType.Exp
            LAG = int(os.environ.get("ATTN_LAG", "2"))

            pending = []  # queue of emitted-S'/exp units awaiting AV emission

            def flush_unit():
                u = pending.pop(0)
                for mmargs in u["av"]:
                    nc.tensor.matmul(**mmargs)
                if u["fin"] is not None:
                    h, j, yps = u["fin"]
                    m, roff = divmod(h, 2)
                    roff *= 64
                    r = work.tile([1, 512], f32, tag="r", bufs=2, name="r")
                    nc.vector.reciprocal(r, yps[64:65, :])
                    rr = work.tile([64, 512], f32, tag="rr", bufs=2, name="rr")
                    nc.gpsimd.partition_broadcast(rr, r)
                    nc.vector.tensor_tensor(
                        yt_sb[roff:roff + 64, m, 512 * j:512 * j + 512],
                        yps[0:64, :], rr, mybir.AluOpType.mult,
                    )

            def attn_head_window(h, j):
                m, roff = divmod(h, 2)
                roff *= 64
                kd_h = kd_sb[roff:roff + 64, m, :]
                qd_h = qd_sb[roff:roff + 64, m, :]
                jwin = slice(512 * j, 512 * (j + 1))
                yps = None
                if attn_mode != "noav":
                    yps = psum.tile([128, 512], f32, tag="y", bufs=2, name="yps")
                nI = 4 * j + 4
                # units of 2 s-chunks -> one [128,1024] exp. Diagonal-block
                # chunks (d = i - 4j >= 0) are narrowed: only columns
                # >= 128*d of the 512-wide q-window are computed (the rest
                # are fully causally masked); the 128-wide triangular mask
                # block is folded into the PE accumulation group.
                for i2 in range(2 * j + 2):
                    sps = psums.tile([128, 1024], f32, tag="s", bufs=3, name="sps")
                    nomask = "nomask" in attn_mode
                    for u in (0, 1):
                        i = 2 * i2 + u
                        d = i - 4 * j  # >= 0 for diagonal-block chunks
                        off = 128 * d if (d > 0 and narrow and not nomask) else 0
                        nc.tensor.matmul(
                            sps[:, 512 * u + off:512 * u + 512],
                            lhsT=kd_h[:, 128 * i:128 * i + 128],
                            rhs=qd_h[:, 512 * j + off:512 * j + 512],
                            start=True, stop=(d < 0 or nomask or dvemask),
                        )
                        if d >= 0 and not nomask and not dvemask:
                            mw = 128 if narrow else 128 * (d + 1)
                            moff = off if narrow else 0
                            nc.tensor.matmul(
                                sps[:, 512 * u + moff:512 * u + moff + mw],
                                lhsT=maskc_sb,                   # identity
                                rhs=maskw_sb[:, d, moff:moff + mw],
                                start=False, stop=True,
                            )
                    pt = work.tile([128, 1024], mm_dt, tag="p", bufs=6, name="pt")
                    f = exp_f if "expcopy" not in attn_mode else \
                        mybir.ActivationFunctionType.Copy
                    offs = []
                    for u in (0, 1):
                        d = 2 * i2 + u - 4 * j
                        offs.append(128 * d if (d > 0 and narrow and not nomask)
                                    else 0)
                    if offs == [0, 0]:
                        nc.scalar.activation(pt, sps, f, scale=0.125)
                    else:
                        for u in (0, 1):
                            sl = slice(512 * u + offs[u], 512 * u + 512)
                            nc.scalar.activation(pt[:, sl], sps[:, sl], f,
                                                 scale=0.125)
                    if dvemask and not nomask:
                        # zero the upper-triangular part of each diagonal
                        # 128-block of exp(S') (bf16 SBUF multiply by 0/1
                        # tri, 2x DVE mode) instead of adding -3000 in PSUM
                        # via identity matmuls.
                        for u in (0, 1):
                            d = 2 * i2 + u - 4 * j
                            if d >= 0:
                                # tri block sits at cols 128*d of the window;
                                # columns below it are skipped by the
                                # narrowed AV (dvemask requires narrow).
                                o = 512 * u + 128 * d
                                nc.vector.tensor_tensor(
                                    pt[:, o:o + 128], pt[:, o:o + 128],
                                    maskt_sb, mybir.AluOpType.mult,
                                )
                    if attn_mode == "noav":
                        continue
                    av = []
                    for u in (0, 1):
                        i = 2 * i2 + u
                        d = i - 4 * j
                        off = 128 * d if (d > 0 and narrow) else 0
                        av.append(dict(
                            out=yps[0:65, off:512],
                            lhsT=v_sb[:, i, 65 * h:65 * h + 65],
                            rhs=pt[:, 512 * u + off:512 * u + 512],
                            start=(i == 0),
                            stop=(i == nI - 1)))
                    pending.append(dict(
                        av=av, fin=(h, j, yps) if i2 == 2 * j + 1 else None))
                    while len(pending) > LAG:
                        flush_unit()

            def proj_window(j):
                for tt in range(4 * j, 4 * j + 4):
                    pso = psums.tile([128, 1024], f32, tag="s", bufs=3,
                                     name="pso")
                    for n2 in range(2):
                        for kc in range(2):
                            nc.tensor.matmul(
                                pso[:, 512 * n2:512 * n2 + 512],
                                lhsT=yt_sb[:, kc, 128 * tt:128 * tt + 128],
                                rhs=wp_sb[:, kc, 512 * n2:512 * n2 + 512],
                                start=(kc == 0), stop=(kc == 1),
                            )
                    os_sb = ostage.tile([128, C], out_dt, tag="osb", name="os_sb")
                    ceng = nc.gpsimd if copy_eng == "pool" else nc.vector
                    ceng.tensor_copy(os_sb, pso)
                    nc.sync.dma_start(out=out[128 * tt:128 * tt + 128, :],
                                      in_=os_sb)

            hlist = [0, 2, 0, 2] if "evenheads" in attn_mode else list(range(HG))
            if "attn" in phases:
                for j in range(NJ):
                    for hx, h in enumerate(hlist):
                        attn_head_window(h, j)
                        if interleave and hx == 1 and j > 0 and "proj" in phases:
                            proj_window(j - 1)
                    if not interleave and "proj" in phases:
                        while pending:
                            flush_unit()
                        proj_window(j)
                while pending:
                    flush_unit()
                if interleave and "proj" in phases:
                    proj_window(NJ - 1)
            elif "proj" in phases:
                for j in range(NJ):
                    proj_window(j)

    nc.finalize()
    return nc


def make_in_maps(x, w_attn, b_attn, w_proj):
    x = np.asarray(x, dtype=np.float32)
    w_attn = np.asarray(w_attn, dtype=np.float32)
    b_attn = np.asarray(b_attn, dtype=np.float32)
    w_proj = np.asarray(w_proj, dtype=np.float32)

    ident = np.eye(128, dtype=np.float32)
    tri = np.where(np.triu(np.ones((128, 128), bool)), 0.0, -3000.0)
    tri01 = np.triu(np.ones((128, 128), np.float32))
    blocks = [ident]
    for d in range(4):
        blk = np.zeros((128, 512), np.float32)
        blk[:, :128 * d] = -3000.0
        blk[:, 128 * d:128 * d + 128] = tri
        blocks.append(blk)
    blocks.append(tri01)
    mask_np = np.concatenate(blocks, axis=1).astype(BF16)  # [128, 128+2048+128]
    # per-batch / per-head-group pieces computed once, shared across cores
    xTs = [np.ascontiguousarray(x[b].T).astype(BF16) for b in range(B)]
    per_g = []
    for g in range(4):
        cq = slice(0 * C + g * DQ, 0 * C + (g + 1) * DQ)
        ck = slice(1 * C + g * DQ, 1 * C + (g + 1) * DQ)
        cv = slice(2 * C + g * DQ, 2 * C + (g + 1) * DQ)
        bq = b_attn[cq]
        bk = b_attn[ck]
        per_g.append({
            "wq": np.ascontiguousarray(w_attn[:, cq]).astype(BF16),
            "wk": np.ascontiguousarray(w_attn[:, ck]).astype(BF16),
            "wv": np.ascontiguousarray(w_attn[:, cv]).astype(BF16),
            "wp": np.ascontiguousarray(w_proj[g * DQ:(g + 1) * DQ, :]).astype(BF16),
            "bqk": np.stack([bq.reshape(2, 128),
                             bk.reshape(2, 128)]).astype(np.float32),
            "bv": np.broadcast_to(b_attn[cv], (128, DQ)).copy().astype(np.float32),
            "mask": mask_np,
        })
    in_maps = []
    for core in range(NCORES):
        b, g = divmod(core, 4)
        in_maps.append({"xT": xTs[b], **per_g[g]})
    return in_maps


def _get_runner():
    """Compile once and keep a reusable sharded executable (repeated
    kernel() calls skip jit retracing and recompilation)."""
    if "runner" in _NC_CACHE:
        return _NC_CACHE["runner"]
    import jax
    import numpy as _np
    from jax.sharding import Mesh, NamedSharding, PartitionSpec
    from jax.experimental.shard_map import shard_map
    from concourse import bass2jax, mybir

    nc = _NC_CACHE.setdefault("nc", build_nc())
    bass2jax.install_neuronx_cc_hook()
    partition_name = nc.partition_id_tensor.name if nc.partition_id_tensor else None
    in_names, out_names, out_avals, zero_outs = [], [], [], []
    for alloc in nc.m.functions[0].allocations:
        if not isinstance(alloc, mybir.MemoryLocationSet):
            continue
        name = alloc.memorylocations[0].name
        if alloc.kind == "ExternalInput":
            if name != partition_name:
                in_names.append(name)
        elif alloc.kind == "ExternalOutput":
            shape = tuple(alloc.tensor_shape)
            dtype = mybir.dt.np(alloc.dtype)
            out_names.append(name)
            out_avals.append(jax.core.ShapedArray(shape, dtype))
            zero_outs.append(_np.zeros(shape, dtype))
    n_params = len(in_names)
    all_in_names = list(in_names) + list(out_names)
    if partition_name is not None:
        all_in_names.append(partition_name)

    def _body(*args):
        operands = list(args)
        if partition_name is not None:
            operands.append(bass2jax.partition_id_tensor())
        outs = bass2jax._bass_exec_p.bind(
            *operands,
            out_avals=tuple(out_avals),
            in_names=tuple(all_in_names),
            out_names=tuple(out_names),
            lowering_input_output_aliases=(),
            sim_require_finite=True,
            sim_require_nnan=True,
            nc=nc,
        )
        return tuple(outs)

    devices = jax.devices()[:NCORES]
    mesh = Mesh(np.asarray(devices), ("core",))
    in_specs = (PartitionSpec("core"),) * (n_params + len(out_names))
    out_specs = (PartitionSpec("core"),) * len(out_names)
    sharded = jax.jit(shard_map(_body, mesh=mesh, in_specs=in_specs,
                                out_specs=out_specs, check_rep=False),
                      keep_unused=True)
    sharding = NamedSharding(mesh, PartitionSpec("core"))
    concat_zeros = [np.zeros((NCORES * z.shape[0], *z.shape[1:]), z.dtype)
                    for z in zero_outs]
    dev_zero = [jax.device_put(a, sharding) for a in concat_zeros]
    runner = dict(sharded=sharded, in_names=in_names, sharding=sharding,
                  dev_zero=dev_zero, out_names=out_names)
    _NC_CACHE["runner"] = runner
    return runner


def kernel(x, w_attn, b_attn, w_proj, b_proj):
    import jax

    r = _get_runner()
    in_maps = make_in_maps(x, w_attn, b_attn, w_proj)
    concat_in = [np.concatenate([in_maps[c][name] for c in range(NCORES)], axis=0)
                 for name in r["in_names"]]
    dev_in = [jax.device_put(a, r["sharding"]) for a in concat_in]
    outs = r["sharded"](*dev_in, *r["dev_zero"])
    out_full = np.asarray(outs[0])  # [NCORES*T, C]

    b_proj = np.asarray(b_proj, dtype=np.float32)
    out = np.zeros((B, T, C), np.float32)
    for core in range(NCORES):
        b = core // 4
        out[b] += out_full[core * T:(core + 1) * T].astype(np.float32)
    out += b_proj[None, None, :]
    return out


# revision 33
# speedup vs baseline: 356.4174x; 1.0672x over previous
"""Causal self-attention (B=2, T=2048, C=1024, H=16, D=64) on 8 TRN2 NeuronCores.

Sharding (Megatron-style, per the hint): data-parallel over the batch (B=2)
and tensor-parallel over heads (16 heads -> 4 groups of 4). Core c handles
batch b = c // 4 and head group g = c % 4:
  - qkv:    computes x[b] @ w_attn[:, cols-of-its-4-heads]  (column split)
  - attn:   full causal attention for its 4 heads
  - proj:   y_heads @ w_proj[rows-of-its-4-heads]           (row split)
The 4 partial proj outputs per batch are summed on the host (+ b_proj).

Device layout notes:
  - All matmuls run in bf16 (inputs pre-cast/pre-transposed on host), fp32
    PSUM accumulation.
  - Scores are computed transposed: S'[s, t] = (k_s . q_t)/8, so softmax sums
    over s (the partition dim) come for free out of the AV matmul by
    augmenting V with a ones column:  yT_aug = [V | 1]^T @ exp(S').
    Row 64 of yT_aug is the softmax denominator per t.
  - exp has no max-subtraction: logits are O(1) for this input distribution
    (|logit| < ~10), so fp32/bf16 exp is safe and the normalization cancels.
  - Diagonal-window S'/mask/AV matmuls are narrowed to skip fully-masked
    column ranges (exp still runs full-width; the stale columns are never
    read by the narrowed AV).
  - Input DMAs are issued on the ACT queue (SP carries the output DMAs), so
    next-iteration input prefetch does not serialize behind output drain.
  - proj runs one q-window behind attention (proj(j-1) between head 1 and
    head 2 of window j) so the PE never waits for the softmax-normalize
    chain; proj PSUM lives in the "s" ring and its PSUM->SBUF copies run on
    the Pool engine, keeping DVE free for the normalize chain.
  - Partial proj outputs are DMA'd out in bf16 (summed in fp32 on host).
"""

import os
import sys

sys.path.insert(0, "/opt/trn_rl_repo")

import numpy as np
import ml_dtypes

BF16 = ml_dtypes.bfloat16

B, T, C, H, D = 2, 2048, 1024, 16, 64
NCORES = 8
HG = 4          # heads per core
DQ = HG * D     # 256 qkv cols per core
CCH = C // 128  # 8 contraction chunks
NT = T // 128   # 16 token chunks of 128
NJ = T // 512   # 4 token tiles of 512

_NC_CACHE = {}


def build_nc(mm_dtype_name="bfloat16", loop=0, phases=("qkv", "attn", "proj"),
             attn_mode="full", dma_eng="act", copy_eng="dve",
             narrow=True, interleave=True, out_bf16=True, dvemask=True,
             av128=True):
    """loop=0: straight-line (graded path). loop=K>0: wrap the body in a
    device-side For_i repeat-K loop (timing builds only). phases: subset for
    bisection timing builds."""
    import contextlib
    import concourse.bacc as bacc
    import concourse.tile as tile
    from concourse import mybir

    mm_dt = getattr(mybir.dt, mm_dtype_name)
    f32 = mybir.dt.float32
    assert narrow or not dvemask, "dvemask requires narrow"

    nc = bacc.Bacc("TRN2", target_bir_lowering=False, debug=False,
                   num_devices=NCORES)

    xT = nc.dram_tensor("xT", [C, T], mm_dt, kind="ExternalInput")
    wq = nc.dram_tensor("wq", [C, DQ], mm_dt, kind="ExternalInput")
    wk = nc.dram_tensor("wk", [C, DQ], mm_dt, kind="ExternalInput")
    wv = nc.dram_tensor("wv", [C, DQ], mm_dt, kind="ExternalInput")
    wp = nc.dram_tensor("wp", [DQ, C], mm_dt, kind="ExternalInput")
    bqk = nc.dram_tensor("bqk", [2, 2, 128], f32, kind="ExternalInput")  # [q/k, chunk, col]
    bv = nc.dram_tensor("bv", [128, DQ], f32, kind="ExternalInput")      # replicated
    mask = nc.dram_tensor("mask", [128, 128 + 4 * 512 + 128], mm_dt,
                          kind="ExternalInput")
    out_dt = mm_dt if out_bf16 else f32
    out = nc.dram_tensor("out", [T, C], out_dt, kind="ExternalOutput")

    with tile.TileContext(nc) as tc:
        with (
            tc.tile_pool(name="const", bufs=1) as const,
            tc.tile_pool(name="acts", bufs=1) as acts,
            tc.tile_pool(name="work", bufs=4) as work,
            tc.tile_pool(name="ostage", bufs=3) as ostage,
            tc.tile_pool(name="psum", bufs=1, space="PSUM") as psum,
            tc.tile_pool(name="psums", bufs=1, space="PSUM") as psums,
            tc.For_i(0, loop, 1,
                     hint_engines=(mybir.EngineType.PE,
                                   mybir.EngineType.Activation,
                                   mybir.EngineType.DVE,
                                   mybir.EngineType.SP,
                                   mybir.EngineType.Pool))
            if loop else contextlib.nullcontext(),
        ):
            # ---- constants / weights (issued on the ACT DMA queue, ordered
            # so qkv compute can start as soon as its operands land) ----
            wq_sb = const.tile([128, CCH, DQ], mm_dt)
            xT_sb = const.tile([128, CCH, T], mm_dt)
            wk_sb = const.tile([128, CCH, DQ], mm_dt)
            wv_sb = const.tile([128, CCH, DQ], mm_dt)
            wp_sb = const.tile([128, 2, C], mm_dt)
            bqk_sb = const.tile([128, 2, 2, 1], f32)  # [col, q/k, chunk, 1]
            bv_sb = const.tile([128, DQ], f32)
            # mask holds [ident(128) | 4 x 512 additive diag masks | 0/1 tri]
            maskc_sb = const.tile([128, 128], mm_dt)
            maskw_sb = const.tile([128, 4, 512], mm_dt)
            maskt_sb = const.tile([128, 128], mm_dt)

            xT_r = xT.rearrange("(c p) t -> p c t", p=128)
            # xT pieces on the ACT queue, everything else on SP (in parallel;
            # SP's out-DMAs only queue up later in the body).
            ldq = nc.scalar if dma_eng == "act" else nc.sync
            ldw = nc.sync if dma_eng == "act" else nc.scalar

            def xpiece(p):
                tw = slice(512 * p, 512 * p + 512)
                ldq.dma_start(out=xT_sb[:, :, tw], in_=xT_r[:, :, tw])

            ldw.dma_start(out=wq_sb, in_=wq.rearrange("(c p) m -> p c m", p=128))
            xpiece(0)
            ldw.dma_start(out=wk_sb, in_=wk.rearrange("(c p) m -> p c m", p=128))
            ldw.dma_start(out=bqk_sb, in_=bqk.rearrange("a m p -> p a m")[:, :, :, None])
            xpiece(1)
            ldw.dma_start(out=wv_sb, in_=wv.rearrange("(c p) m -> p c m", p=128))
            ldw.dma_start(out=bv_sb, in_=bv[:, :])
            xpiece(2)
            ldw.dma_start(out=wp_sb, in_=wp.rearrange("(k p) n -> p k n", p=128))
            if dvemask:
                ldw.dma_start(out=maskt_sb, in_=mask[:, 128 + 2048:])
            else:
                ldw.dma_start(out=maskc_sb, in_=mask[:, 0:128])
                ldw.dma_start(out=maskw_sb,
                              in_=mask[:, 128:128 + 2048].rearrange(
                                  "p (a n) -> p a n", a=4))
            xpiece(3)

            # ---- activations ----
            qd_sb = acts.tile([128, 2, T], mm_dt)   # [dcol, chunk, t]
            kd_sb = acts.tile([128, 2, T], mm_dt)
            # per s-chunk: 4 head slots of [V_h | 1 | pad]; av128 pads the
            # slot stride so the AV lhsT can be a full 128 columns.
            SL = 88 if av128 else 65
            vw = SL * 3 + 128 if av128 else HG * 65
            v_sb = acts.tile([128, NT, vw], mm_dt)
            yt_sb = acts.tile([128, 2, T], mm_dt)

            if av128:
                nc.vector.memset(v_sb, 0.0)
            # ones columns of v_sb (col 64 of each head slot)
            ones_view = v_sb[:, :, 0:4 * SL].rearrange(
                "p s (h e) -> p s h e", e=SL)[:, :, :, 64:65]
            nc.vector.memset(ones_view, 1.0)

            # bisection timing builds: initialize tensors a skipped phase
            # would have produced
            if "qkv" not in phases:
                nc.vector.memset(qd_sb, 0.5)
                nc.vector.memset(kd_sb, 0.5)
                nc.vector.memset(v_sb, 0.5)
            if "attn" not in phases:
                nc.vector.memset(yt_sb, 0.5)

            # ---- phase 1: qkv projections ----
            # Qd/Kd in d-major [dcol, t]; out tile = W_chunk^T @ xT_chunk.
            # Emission order (m=0 Q, m=0 K, V, m=1 Q, m=1 K) lets heads 0/1
            # attention start while heads 2/3 qkv still runs.
            def qk_proj_j(dst, wsb, qki, m, j):
                ps = psum.tile([128, 512], f32, tag="y", bufs=2, name="ps_qk")
                for c in range(CCH):
                    nc.tensor.matmul(
                        ps,
                        lhsT=wsb[:, c, 128 * m:128 * m + 128],
                        rhs=xT_sb[:, c, 512 * j:512 * j + 512],
                        start=(c == 0), stop=(c == CCH - 1),
                    )
                nc.vector.tensor_scalar_add(
                    dst[:, m, 512 * j:512 * j + 512], ps,
                    bqk_sb[:, qki, m, :],
                )

            def v_proj_tt(tt):
                # V in s-major [t, vcol]; out tile = xT_chunk(t)^T @ Wv_chunk
                ps = psum.tile([128, 512], f32, tag="y", bufs=2, name="ps_v")
                for c in range(CCH):
                    nc.tensor.matmul(
                        ps[:, 0:DQ],
                        lhsT=xT_sb[:, c, 128 * tt:128 * tt + 128],
                        rhs=wv_sb[:, c, :],
                        start=(c == 0), stop=(c == CCH - 1),
                    )
                nc.vector.tensor_tensor(
                    v_sb[:, :, 0:4 * SL].rearrange(
                        "p s (h e) -> p s h e", e=SL)[:, tt, :, 0:64],
                    ps[:, 0:DQ].rearrange("p (h d) -> p h d", d=64),
                    bv_sb.rearrange("p (h d) -> p h d", d=64),
                    mybir.AluOpType.add,
                )

            if "qkv" in phases:
                # piece-interleaved: q/k/v for xT piece p emitted together so
                # PE work rate-matches the xT piece DMAs at iteration start
                for j in range(NJ):
                    qk_proj_j(qd_sb, wq_sb, 0, 0, j)
                    qk_proj_j(kd_sb, wk_sb, 1, 0, j)
                    for tt in range(4 * j, 4 * j + 4):
                        v_proj_tt(tt)
                for j in range(NJ):
                    qk_proj_j(qd_sb, wq_sb, 0, 1, j)
                    qk_proj_j(kd_sb, wk_sb, 1, 1, j)

            # ---- phase 2+3: attention (j outer, h inner) with proj lagging
            # one window behind (proj(j-1) emitted between head 1 and head 2
            # of window j). Software-pipelined AV emission: AV of unit k is
            # emitted after the S' matmuls of unit k+LAG, so the in-order PE
            # stream never blocks on the ~1.2us ACT exp latency.
            exp_f = mybir.ActivationFunctionType.Exp
            LAG = int(os.environ.get("ATTN_LAG", "2"))

            pending = []  # queue of emitted-S'/exp units awaiting AV emission

            def flush_unit():
                u = pending.pop(0)
                for mmargs in u["av"]:
                    nc.tensor.matmul(**mmargs)
                if u["fin"] is not None and "nofin" not in attn_mode:
                    h, j, yps = u["fin"]
                    m, roff = divmod(h, 2)
                    roff *= 64
                    r = work.tile([1, 512], f32, tag="r", bufs=2, name="r")
                    nc.vector.reciprocal(r, yps[64:65, :])
                    rr = work.tile([64, 512], f32, tag="rr", bufs=2, name="rr")
                    nc.gpsimd.partition_broadcast(rr, r)
                    nc.vector.tensor_tensor(
                        yt_sb[roff:roff + 64, m, 512 * j:512 * j + 512],
                        yps[0:64, :], rr, mybir.AluOpType.mult,
                    )

            def attn_head_window(h, j):
                m, roff = divmod(h, 2)
                roff *= 64
                kd_h = kd_sb[roff:roff + 64, m, :]
                qd_h = qd_sb[roff:roff + 64, m, :]
                jwin = slice(512 * j, 512 * (j + 1))
                yps = None
                if attn_mode != "noav":
                    yps = psum.tile([128, 512], f32, tag="y", bufs=2, name="yps")
                nI = 4 * j + 4
                # units of 2 s-chunks -> one [128,1024] exp. Diagonal-block
                # chunks (d = i - 4j >= 0) are narrowed: only columns
                # >= 128*d of the 512-wide q-window are computed (the rest
                # are fully causally masked); the 128-wide triangular mask
                # block is folded into the PE accumulation group.
                for i2 in range(2 * j + 2):
                    sps = psums.tile([128, 1024], f32, tag="s", bufs=3, name="sps")
                    nomask = "nomask" in attn_mode
                    for u in (0, 1):
                        i = 2 * i2 + u
                        d = i - 4 * j  # >= 0 for diagonal-block chunks
                        off = 128 * d if (d > 0 and narrow and not nomask) else 0
                        nc.tensor.matmul(
                            sps[:, 512 * u + off:512 * u + 512],
                            lhsT=kd_h[:, 128 * i:128 * i + 128],
                            rhs=qd_h[:, 512 * j + off:512 * j + 512],
                            start=True, stop=(d < 0 or nomask or dvemask),
                        )
                        if d >= 0 and not nomask and not dvemask:
                            mw = 128 if narrow else 128 * (d + 1)
                            moff = off if narrow else 0
                            nc.tensor.matmul(
                                sps[:, 512 * u + moff:512 * u + moff + mw],
                                lhsT=maskc_sb,                   # identity
                                rhs=maskw_sb[:, d, moff:moff + mw],
                                start=False, stop=True,
                            )
                    pt = work.tile([128, 1024], mm_dt, tag="p", bufs=6, name="pt")
                    f = exp_f if "expcopy" not in attn_mode else \
                        mybir.ActivationFunctionType.Copy
                    offs = []
                    for u in (0, 1):
                        d = 2 * i2 + u - 4 * j
                        offs.append(128 * d if (d > 0 and narrow and not nomask)
                                    else 0)
                    if offs == [0, 0]:
                        nc.scalar.activation(pt, sps, f, scale=0.125)
                    else:
                        for u in (0, 1):
                            sl = slice(512 * u + offs[u], 512 * u + 512)
                            nc.scalar.activation(pt[:, sl], sps[:, sl], f,
                                                 scale=0.125)
                    if dvemask and not nomask:
                        # zero the upper-triangular part of each diagonal
                        # 128-block of exp(S') (bf16 SBUF multiply by 0/1
                        # tri, 2x DVE mode) instead of adding -3000 in PSUM
                        # via identity matmuls.
                        for u in (0, 1):
                            d = 2 * i2 + u - 4 * j
                            if d >= 0:
                                # tri block sits at cols 128*d of the window;
                                # columns below it are skipped by the
                                # narrowed AV (dvemask requires narrow).
                                o = 512 * u + 128 * d
                                nc.vector.tensor_tensor(
                                    pt[:, o:o + 128], pt[:, o:o + 128],
                                    maskt_sb, mybir.AluOpType.mult,
                                )
                    if attn_mode == "noav":
                        continue
                    av = []
                    for u in (0, 1):
                        i = 2 * i2 + u
                        d = i - 4 * j
                        off = 128 * d if (d > 0 and narrow) else 0
                        lw = 128 if av128 else 65
                        av.append(dict(
                            out=yps[0:lw, off:512],
                            lhsT=v_sb[:, i, SL * h:SL * h + lw],
                            rhs=pt[:, 512 * u + off:512 * u + 512],
                            start=(i == 0),
                            stop=(i == nI - 1)))
                    pending.append(dict(
                        av=av, fin=(h, j, yps) if i2 == 2 * j + 1 else None))
                    while len(pending) > LAG:
                        flush_unit()

            def proj_window(j):
                for tt in range(4 * j, 4 * j + 4):
                    pso = psums.tile([128, 1024], f32, tag="s", bufs=3,
                                     name="pso")
                    for n2 in range(2):
                        for kc in range(2):
                            nc.tensor.matmul(
                                pso[:, 512 * n2:512 * n2 + 512],
                                lhsT=yt_sb[:, kc, 128 * tt:128 * tt + 128],
                                rhs=wp_sb[:, kc, 512 * n2:512 * n2 + 512],
                                start=(kc == 0), stop=(kc == 1),
                            )
                    os_sb = ostage.tile([128, C], out_dt, tag="osb", name="os_sb")
                    ceng = nc.gpsimd if copy_eng == "pool" else nc.vector
                    ceng.tensor_copy(os_sb, pso)
                    nc.sync.dma_start(out=out[128 * tt:128 * tt + 128, :],
                                      in_=os_sb)

            hlist = [0, 2, 0, 2] if "evenheads" in attn_mode else list(range(HG))
            if "attn" in phases:
                for j in range(NJ):
                    for hx, h in enumerate(hlist):
                        attn_head_window(h, j)
                        if interleave and hx == 1 and j > 0 and "proj" in phases:
                            proj_window(j - 1)
                    if not interleave and "proj" in phases:
                        while pending:
                            flush_unit()
                        proj_window(j)
                while pending:
                    flush_unit()
                if interleave and "proj" in phases:
                    proj_window(NJ - 1)
            elif "proj" in phases:
                for j in range(NJ):
                    proj_window(j)

    nc.finalize()
    return nc


def make_in_maps(x, w_attn, b_attn, w_proj):
    x = np.asarray(x, dtype=np.float32)
    w_attn = np.asarray(w_attn, dtype=np.float32)
    b_attn = np.asarray(b_attn, dtype=np.float32)
    w_proj = np.asarray(w_proj, dtype=np.float32)

    ident = np.eye(128, dtype=np.float32)
    tri = np.where(np.triu(np.ones((128, 128), bool)), 0.0, -3000.0)
    tri01 = np.triu(np.ones((128, 128), np.float32))
    blocks = [ident]
    for d in range(4):
        blk = np.zeros((128, 512), np.float32)
        blk[:, :128 * d] = -3000.0
        blk[:, 128 * d:128 * d + 128] = tri
        blocks.append(blk)
    blocks.append(tri01)
    mask_np = np.concatenate(blocks, axis=1).astype(BF16)  # [128, 128+2048+128]
    # per-batch / per-head-group pieces computed once, shared across cores
    xTs = [np.ascontiguousarray(x[b].T).astype(BF16) for b in range(B)]
    per_g = []
    for g in range(4):
        cq = slice(0 * C + g * DQ, 0 * C + (g + 1) * DQ)
        ck = slice(1 * C + g * DQ, 1 * C + (g + 1) * DQ)
        cv = slice(2 * C + g * DQ, 2 * C + (g + 1) * DQ)
        bq = b_attn[cq]
        bk = b_attn[ck]
        per_g.append({
            "wq": np.ascontiguousarray(w_attn[:, cq]).astype(BF16),
            "wk": np.ascontiguousarray(w_attn[:, ck]).astype(BF16),
            "wv": np.ascontiguousarray(w_attn[:, cv]).astype(BF16),
            "wp": np.ascontiguousarray(w_proj[g * DQ:(g + 1) * DQ, :]).astype(BF16),
            "bqk": np.stack([bq.reshape(2, 128),
                             bk.reshape(2, 128)]).astype(np.float32),
            "bv": np.broadcast_to(b_attn[cv], (128, DQ)).copy().astype(np.float32),
            "mask": mask_np,
        })
    in_maps = []
    for core in range(NCORES):
        b, g = divmod(core, 4)
        in_maps.append({"xT": xTs[b], **per_g[g]})
    return in_maps


def _get_runner():
    """Compile once and keep a reusable sharded executable (repeated
    kernel() calls skip jit retracing and recompilation)."""
    if "runner" in _NC_CACHE:
        return _NC_CACHE["runner"]
    import jax
    import numpy as _np
    from jax.sharding import Mesh, NamedSharding, PartitionSpec
    from jax.experimental.shard_map import shard_map
    from concourse import bass2jax, mybir

    nc = _NC_CACHE.setdefault("nc", build_nc())
    bass2jax.install_neuronx_cc_hook()
    partition_name = nc.partition_id_tensor.name if nc.partition_id_tensor else None
    in_names, out_names, out_avals, zero_outs = [], [], [], []
    for alloc in nc.m.functions[0].allocations:
        if not isinstance(alloc, mybir.MemoryLocationSet):
            continue
        name = alloc.memorylocations[0].name
        if alloc.kind == "ExternalInput":
            if name != partition_name:
                in_names.append(name)
        elif alloc.kind == "ExternalOutput":
            shape = tuple(alloc.tensor_shape)
            dtype = mybir.dt.np(alloc.dtype)
            out_names.append(name)
            out_avals.append(jax.core.ShapedArray(shape, dtype))
            zero_outs.append(_np.zeros(shape, dtype))
    n_params = len(in_names)
    all_in_names = list(in_names) + list(out_names)
    if partition_name is not None:
        all_in_names.append(partition_name)

    def _body(*args):
        operands = list(args)
        if partition_name is not None:
            operands.append(bass2jax.partition_id_tensor())
        outs = bass2jax._bass_exec_p.bind(
            *operands,
            out_avals=tuple(out_avals),
            in_names=tuple(all_in_names),
            out_names=tuple(out_names),
            lowering_input_output_aliases=(),
            sim_require_finite=True,
            sim_require_nnan=True,
            nc=nc,
        )
        return tuple(outs)

    devices = jax.devices()[:NCORES]
    mesh = Mesh(np.asarray(devices), ("core",))
    in_specs = (PartitionSpec("core"),) * (n_params + len(out_names))
    out_specs = (PartitionSpec("core"),) * len(out_names)
    sharded = jax.jit(shard_map(_body, mesh=mesh, in_specs=in_specs,
                                out_specs=out_specs, check_rep=False),
                      keep_unused=True)
    sharding = NamedSharding(mesh, PartitionSpec("core"))
    concat_zeros = [np.zeros((NCORES * z.shape[0], *z.shape[1:]), z.dtype)
                    for z in zero_outs]
    dev_zero = [jax.device_put(a, sharding) for a in concat_zeros]
    runner = dict(sharded=sharded, in_names=in_names, sharding=sharding,
                  dev_zero=dev_zero, out_names=out_names)
    _NC_CACHE["runner"] = runner
    return runner


def kernel(x, w_attn, b_attn, w_proj, b_proj):
    import jax

    r = _get_runner()
    in_maps = make_in_maps(x, w_attn, b_attn, w_proj)
    concat_in = [np.concatenate([in_maps[c][name] for c in range(NCORES)], axis=0)
                 for name in r["in_names"]]
    dev_in = [jax.device_put(a, r["sharding"]) for a in concat_in]
    outs = r["sharded"](*dev_in, *r["dev_zero"])
    out_full = np.asarray(outs[0])  # [NCORES*T, C]

    b_proj = np.asarray(b_proj, dtype=np.float32)
    out = np.zeros((B, T, C), np.float32)
    for core in range(NCORES):
        b = core // 4
        out[b] += out_full[core * T:(core + 1) * T].astype(np.float32)
    out += b_proj[None, None, :]
    return out


# revision 44
# speedup vs baseline: 380.4562x; 1.0674x over previous
"""Causal self-attention (B=2, T=2048, C=1024, H=16, D=64) on 8 TRN2 NeuronCores.

Sharding (Megatron-style, per the hint): data-parallel over the batch (B=2)
and tensor-parallel over heads (16 heads -> 4 groups of 4). Core c handles
batch b = c // 4 and head group g = c % 4:
  - qkv:    computes x[b] @ w_attn[:, cols-of-its-4-heads]  (column split)
  - attn:   full causal attention for its 4 heads
  - proj:   y_heads @ w_proj[rows-of-its-4-heads]           (row split)
The 4 partial proj outputs per batch are summed on the host (+ b_proj).

Device layout notes:
  - All matmuls run in bf16 (inputs pre-cast/pre-transposed on host), fp32
    PSUM accumulation.
  - Scores are computed transposed: S'[s, t] = (k_s . q_t)/8, so softmax sums
    over s (the partition dim) come for free out of the AV matmul by
    augmenting V with a ones column:  yT_aug = [V | 1]^T @ exp(S').
    Row 64 of yT_aug is the softmax denominator per t.
  - exp has no max-subtraction: logits are O(1) for this input distribution
    (|logit| < ~10), so fp32/bf16 exp is safe and the normalization cancels.
  - Diagonal-window S'/mask/AV matmuls are narrowed to skip fully-masked
    column ranges (exp still runs full-width; the stale columns are never
    read by the narrowed AV).
  - Input DMAs are issued on the ACT queue (SP carries the output DMAs), so
    next-iteration input prefetch does not serialize behind output drain.
  - proj runs one q-window behind attention (proj(j-1) between head 1 and
    head 2 of window j) so the PE never waits for the softmax-normalize
    chain; proj PSUM lives in the "s" ring and its PSUM->SBUF copies run on
    the Pool engine, keeping DVE free for the normalize chain.
  - Partial proj outputs are DMA'd out in bf16 (summed in fp32 on host).
"""

import os
import sys

sys.path.insert(0, "/opt/trn_rl_repo")

import numpy as np
import ml_dtypes

BF16 = ml_dtypes.bfloat16

B, T, C, H, D = 2, 2048, 1024, 16, 64
NCORES = 8
HG = 4          # heads per core
DQ = HG * D     # 256 qkv cols per core
CCH = C // 128  # 8 contraction chunks
NT = T // 128   # 16 token chunks of 128
NJ = T // 512   # 4 token tiles of 512

_NC_CACHE = {}


def build_nc(mm_dtype_name="bfloat16", loop=0, phases=("qkv", "attn", "proj"),
             attn_mode="full_psplit", dma_eng="act", copy_eng="dve",
             narrow=True, interleave=True, out_bf16=True, dvemask=True,
             av128=True, ybufs=2, sbufs=3, finsb=False):
    """loop=0: straight-line (graded path). loop=K>0: wrap the body in a
    device-side For_i repeat-K loop (timing builds only). phases: subset for
    bisection timing builds."""
    import contextlib
    import concourse.bacc as bacc
    import concourse.tile as tile
    from concourse import mybir

    mm_dt = getattr(mybir.dt, mm_dtype_name)
    f32 = mybir.dt.float32
    assert narrow or not dvemask, "dvemask requires narrow"

    nc = bacc.Bacc("TRN2", target_bir_lowering=False, debug=False,
                   num_devices=NCORES)

    xT = nc.dram_tensor("xT", [C, T], mm_dt, kind="ExternalInput")
    wq = nc.dram_tensor("wq", [C, DQ], mm_dt, kind="ExternalInput")
    wk = nc.dram_tensor("wk", [C, DQ], mm_dt, kind="ExternalInput")
    wv = nc.dram_tensor("wv", [C, DQ], mm_dt, kind="ExternalInput")
    wp = nc.dram_tensor("wp", [DQ, C], mm_dt, kind="ExternalInput")
    bqk = nc.dram_tensor("bqk", [2, 2, 128], f32, kind="ExternalInput")  # [q/k, chunk, col]
    bv = nc.dram_tensor("bv", [128, DQ], f32, kind="ExternalInput")      # replicated
    mask = nc.dram_tensor("mask", [128, 128 + 4 * 512 + 128], mm_dt,
                          kind="ExternalInput")
    out_dt = mm_dt if out_bf16 else f32
    out = nc.dram_tensor("out", [T, C], out_dt, kind="ExternalOutput")

    with tile.TileContext(nc) as tc:
        with (
            tc.tile_pool(name="const", bufs=1) as const,
            tc.tile_pool(name="acts", bufs=1) as acts,
            tc.tile_pool(name="work", bufs=4) as work,
            tc.tile_pool(name="ostage", bufs=3) as ostage,
            tc.tile_pool(name="psum", bufs=1, space="PSUM") as psum,
            tc.tile_pool(name="psums", bufs=1, space="PSUM") as psums,
            tc.For_i(0, loop, 1,
                     hint_engines=(mybir.EngineType.PE,
                                   mybir.EngineType.Activation,
                                   mybir.EngineType.DVE,
                                   mybir.EngineType.SP,
                                   mybir.EngineType.Pool))
            if loop else contextlib.nullcontext(),
        ):
            # ---- constants / weights (issued on the ACT DMA queue, ordered
            # so qkv compute can start as soon as its operands land) ----
            wq_sb = const.tile([128, CCH, DQ], mm_dt)
            xT_sb = const.tile([128, CCH, T], mm_dt)
            wk_sb = const.tile([128, CCH, DQ], mm_dt)
            wv_sb = const.tile([128, CCH, DQ], mm_dt)
            wp_sb = const.tile([128, 2, C], mm_dt)
            bqk_sb = const.tile([128, 2, 2, 1], f32)  # [col, q/k, chunk, 1]
            bv_sb = const.tile([128, DQ], f32)
            # mask holds [ident(128) | 4 x 512 additive diag masks | 0/1 tri]
            maskc_sb = const.tile([128, 128], mm_dt)
            maskw_sb = const.tile([128, 4, 512], mm_dt)
            maskt_sb = const.tile([128, 128], mm_dt)

            xT_r = xT.rearrange("(c p) t -> p c t", p=128)
            # xT pieces on the ACT queue, everything else on SP (in parallel;
            # SP's out-DMAs only queue up later in the body).
            ldq = nc.scalar if dma_eng == "act" else nc.sync
            ldw = nc.sync if dma_eng == "act" else nc.scalar

            def xpiece(p):
                tw = slice(512 * p, 512 * p + 512)
                ldq.dma_start(out=xT_sb[:, :, tw], in_=xT_r[:, :, tw])

            ldw.dma_start(out=wq_sb, in_=wq.rearrange("(c p) m -> p c m", p=128))
            xpiece(0)
            ldw.dma_start(out=wk_sb, in_=wk.rearrange("(c p) m -> p c m", p=128))
            ldw.dma_start(out=bqk_sb, in_=bqk.rearrange("a m p -> p a m")[:, :, :, None])
            xpiece(1)
            ldw.dma_start(out=wv_sb, in_=wv.rearrange("(c p) m -> p c m", p=128))
            ldw.dma_start(out=bv_sb, in_=bv[:, :])
            xpiece(2)
            ldw.dma_start(out=wp_sb, in_=wp.rearrange("(k p) n -> p k n", p=128))
            if dvemask:
                ldw.dma_start(out=maskt_sb, in_=mask[:, 128 + 2048:])
            else:
                ldw.dma_start(out=maskc_sb, in_=mask[:, 0:128])
                ldw.dma_start(out=maskw_sb,
                              in_=mask[:, 128:128 + 2048].rearrange(
                                  "p (a n) -> p a n", a=4))
            xpiece(3)

            # ---- activations ----
            qd_sb = acts.tile([128, 2, T], mm_dt)   # [dcol, chunk, t]
            kd_sb = acts.tile([128, 2, T], mm_dt)
            # per s-chunk: 4 head slots of [V_h | 1 | pad]; av128 pads the
            # slot stride so the AV lhsT can be a full 128 columns.
            SL = 88 if av128 else 65
            vw = SL * 3 + 128 if av128 else HG * 65
            v_sb = acts.tile([128, NT, vw], mm_dt)
            yt_sb = acts.tile([128, 2, T], mm_dt)

            if av128:
                nc.vector.memset(v_sb, 0.0)
            # ones columns of v_sb (col 64 of each head slot)
            ones_view = v_sb[:, :, 0:4 * SL].rearrange(
                "p s (h e) -> p s h e", e=SL)[:, :, :, 64:65]
            nc.vector.memset(ones_view, 1.0)

            # bisection timing builds: initialize tensors a skipped phase
            # would have produced
            if "qkv" not in phases:
                nc.vector.memset(qd_sb, 0.5)
                nc.vector.memset(kd_sb, 0.5)
                nc.vector.memset(v_sb, 0.5)
            if "attn" not in phases:
                nc.vector.memset(yt_sb, 0.5)

            # ---- phase 1: qkv projections ----
            # Qd/Kd in d-major [dcol, t]; out tile = W_chunk^T @ xT_chunk.
            # Emission order (m=0 Q, m=0 K, V, m=1 Q, m=1 K) lets heads 0/1
            # attention start while heads 2/3 qkv still runs.
            def qk_proj_j(dst, wsb, qki, m, j):
                ps = psum.tile([128, 512], f32, tag="y", bufs=ybufs, name="ps_qk")
                for c in range(CCH):
                    nc.tensor.matmul(
                        ps,
                        lhsT=wsb[:, c, 128 * m:128 * m + 128],
                        rhs=xT_sb[:, c, 512 * j:512 * j + 512],
                        start=(c == 0), stop=(c == CCH - 1),
                    )
                nc.vector.tensor_scalar_add(
                    dst[:, m, 512 * j:512 * j + 512], ps,
                    bqk_sb[:, qki, m, :],
                )

            def v_proj_tt(tt):
                # V in s-major [t, vcol]; out tile = xT_chunk(t)^T @ Wv_chunk
                ps = psum.tile([128, 512], f32, tag="y", bufs=ybufs, name="ps_v")
                for c in range(CCH):
                    nc.tensor.matmul(
                        ps[:, 0:DQ],
                        lhsT=xT_sb[:, c, 128 * tt:128 * tt + 128],
                        rhs=wv_sb[:, c, :],
                        start=(c == 0), stop=(c == CCH - 1),
                    )
                nc.vector.tensor_tensor(
                    v_sb[:, :, 0:4 * SL].rearrange(
                        "p s (h e) -> p s h e", e=SL)[:, tt, :, 0:64],
                    ps[:, 0:DQ].rearrange("p (h d) -> p h d", d=64),
                    bv_sb.rearrange("p (h d) -> p h d", d=64),
                    mybir.AluOpType.add,
                )

            if "qkv" in phases:
                # piece-interleaved: q/k/v for xT piece p emitted together so
                # PE work rate-matches the xT piece DMAs at iteration start
                for j in range(NJ):
                    qk_proj_j(qd_sb, wq_sb, 0, 0, j)
                    qk_proj_j(kd_sb, wk_sb, 1, 0, j)
                    for tt in range(4 * j, 4 * j + 4):
                        v_proj_tt(tt)
                for j in range(NJ):
                    qk_proj_j(qd_sb, wq_sb, 0, 1, j)
                    qk_proj_j(kd_sb, wk_sb, 1, 1, j)

            # ---- phase 2+3: attention (j outer, h inner) with proj lagging
            # one window behind (proj(j-1) emitted between head 1 and head 2
            # of window j). Software-pipelined AV emission: AV of unit k is
            # emitted after the S' matmuls of unit k+LAG, so the in-order PE
            # stream never blocks on the ~1.2us ACT exp latency.
            exp_f = mybir.ActivationFunctionType.Exp
            LAG = int(os.environ.get("ATTN_LAG", "3"))

            pending = []  # queue of emitted-S'/exp units awaiting AV emission

            def flush_unit():
                u = pending.pop(0)
                for mmargs in u["av"]:
                    nc.tensor.matmul(**mmargs)
                if u["fin"] is not None and "nofin" not in attn_mode:
                    h, j, yps = u["fin"]
                    m, roff = divmod(h, 2)
                    roff *= 64
                    if finsb:
                        # stage yps to SBUF with one copy (frees the PSUM
                        # bank early), then run the whole normalize chain
                        # SBUF-only with broadcast+mult on Pool.
                        ya = work.tile([65, 512], f32, tag="ya", bufs=3,
                                       name="ya")
                        nc.vector.tensor_copy(ya, yps[0:65, :])
                        r = work.tile([1, 512], f32, tag="r", bufs=2, name="r")
                        nc.vector.reciprocal(r, ya[64:65, :])
                        rr = work.tile([64, 512], f32, tag="rr", bufs=2,
                                       name="rr")
                        nc.gpsimd.partition_broadcast(rr, r)
                        nc.gpsimd.tensor_tensor(
                            yt_sb[roff:roff + 64, m, 512 * j:512 * j + 512],
                            ya[0:64, :], rr, mybir.AluOpType.mult,
                        )
                    else:
                        r = work.tile([1, 512], f32, tag="r", bufs=2, name="r")
                        nc.vector.reciprocal(r, yps[64:65, :])
                        rr = work.tile([64, 512], f32, tag="rr", bufs=2,
                                       name="rr")
                        nc.gpsimd.partition_broadcast(rr, r)
                        nc.vector.tensor_tensor(
                            yt_sb[roff:roff + 64, m, 512 * j:512 * j + 512],
                            yps[0:64, :], rr, mybir.AluOpType.mult,
                        )

            def attn_head_window(h, j):
                m, roff = divmod(h, 2)
                roff *= 64
                kd_h = kd_sb[roff:roff + 64, m, :]
                qd_h = qd_sb[roff:roff + 64, m, :]
                jwin = slice(512 * j, 512 * (j + 1))
                yps = None
                if attn_mode != "noav":
                    yps = psum.tile([128, 512], f32, tag="y", bufs=ybufs,
                                    name="yps")
                nI = 4 * j + 4
                # units of 2 s-chunks -> one [128,1024] exp. Diagonal-block
                # chunks (d = i - 4j >= 0) are narrowed: only columns
                # >= 128*d of the 512-wide q-window are computed (the rest
                # are fully causally masked); the 128-wide triangular mask
                # block is folded into the PE accumulation group.
                for i2 in range(2 * j + 2):
                    sps = psums.tile([128, 1024], f32, tag="s", bufs=sbufs,
                                     name="sps")
                    nomask = "nomask" in attn_mode
                    for u in (0, 1):
                        i = 2 * i2 + u
                        d = i - 4 * j  # >= 0 for diagonal-block chunks
                        off = 128 * d if (d > 0 and narrow and not nomask) else 0
                        nc.tensor.matmul(
                            sps[:, 512 * u + off:512 * u + 512],
                            lhsT=kd_h[:, 128 * i:128 * i + 128],
                            rhs=qd_h[:, 512 * j + off:512 * j + 512],
                            start=True, stop=(d < 0 or nomask or dvemask),
                        )
                        if d >= 0 and not nomask and not dvemask:
                            mw = 128 if narrow else 128 * (d + 1)
                            moff = off if narrow else 0
                            nc.tensor.matmul(
                                sps[:, 512 * u + moff:512 * u + moff + mw],
                                lhsT=maskc_sb,                   # identity
                                rhs=maskw_sb[:, d, moff:moff + mw],
                                start=False, stop=True,
                            )
                    pt = work.tile([128, 1024], mm_dt, tag="p", bufs=6, name="pt")
                    f = exp_f if "expcopy" not in attn_mode else \
                        mybir.ActivationFunctionType.Copy
                    offs = []
                    for u in (0, 1):
                        d = 2 * i2 + u - 4 * j
                        offs.append(128 * d if (d > 0 and narrow and not nomask)
                                    else 0)
                    if offs == [0, 0]:
                        nc.scalar.activation(pt, sps, f, scale=0.125)
                    else:
                        for u in (0, 1):
                            sl = slice(512 * u + offs[u], 512 * u + 512)
                            nc.scalar.activation(pt[:, sl], sps[:, sl], f,
                                                 scale=0.125)
                    if dvemask and not nomask:
                        # zero the upper-triangular part of each diagonal
                        # 128-block of exp(S') (bf16 SBUF multiply by 0/1
                        # tri, 2x DVE mode) instead of adding -3000 in PSUM
                        # via identity matmuls.
                        for u in (0, 1):
                            d = 2 * i2 + u - 4 * j
                            if d >= 0:
                                # tri block sits at cols 128*d of the window;
                                # columns below it are skipped by the
                                # narrowed AV (dvemask requires narrow).
                                o = 512 * u + 128 * d
                                nc.vector.tensor_tensor(
                                    pt[:, o:o + 128], pt[:, o:o + 128],
                                    maskt_sb, mybir.AluOpType.mult,
                                )
                    if attn_mode == "noav":
                        continue
                    av = []
                    for u in (0, 1):
                        i = 2 * i2 + u
                        d = i - 4 * j
                        off = 128 * d if (d > 0 and narrow) else 0
                        lw = 128 if av128 else 65
                        av.append(dict(
                            out=yps[0:lw, off:512],
                            lhsT=v_sb[:, i, SL * h:SL * h + lw],
                            rhs=pt[:, 512 * u + off:512 * u + 512],
                            start=(i == 0),
                            stop=(i == nI - 1)))
                    pending.append(dict(
                        av=av, fin=(h, j, yps) if i2 == 2 * j + 1 else None))
                    while len(pending) > LAG:
                        flush_unit()

            def proj_window(j, half=None):
                tts = range(4 * j, 4 * j + 4)
                if half is not None:
                    tts = tts[:2] if half == 0 else tts[2:]
                for tt in tts:
                    pso = psums.tile([128, 1024], f32, tag="s", bufs=sbufs,
                                     name="pso")
                    for n2 in range(2):
                        for kc in range(2):
                            nc.tensor.matmul(
                                pso[:, 512 * n2:512 * n2 + 512],
                                lhsT=yt_sb[:, kc, 128 * tt:128 * tt + 128],
                                rhs=wp_sb[:, kc, 512 * n2:512 * n2 + 512],
                                start=(kc == 0), stop=(kc == 1),
                            )
                    os_sb = ostage.tile([128, C], out_dt, tag="osb", name="os_sb")
                    ceng = nc.gpsimd if copy_eng == "pool" else nc.vector
                    ceng.tensor_copy(os_sb, pso)
                    nc.sync.dma_start(out=out[128 * tt:128 * tt + 128, :],
                                      in_=os_sb)

            hlist = [0, 2, 0, 2] if "evenheads" in attn_mode else list(range(HG))
            if "attn" in phases:
                psplit = "psplit" in attn_mode
                for j in range(NJ):
                    for hx, h in enumerate(hlist):
                        attn_head_window(h, j)
                        if interleave and j > 0 and "proj" in phases:
                            if psplit and hx in (1, 2):
                                proj_window(j - 1, half=hx - 1)
                            elif not psplit and hx == 1:
                                proj_window(j - 1)
                    if not interleave and "proj" in phases:
                        while pending:
                            flush_unit()
                        proj_window(j)
                while pending:
                    flush_unit()
                if interleave and "proj" in phases:
                    proj_window(NJ - 1)
            elif "proj" in phases:
                for j in range(NJ):
                    proj_window(j)

    nc.finalize()
    return nc


def make_in_maps(x, w_attn, b_attn, w_proj):
    x = np.asarray(x, dtype=np.float32)
    w_attn = np.asarray(w_attn, dtype=np.float32)
    b_attn = np.asarray(b_attn, dtype=np.float32)
    w_proj = np.asarray(w_proj, dtype=np.float32)

    ident = np.eye(128, dtype=np.float32)
    tri = np.where(np.triu(np.ones((128, 128), bool)), 0.0, -3000.0)
    tri01 = np.triu(np.ones((128, 128), np.float32))
    blocks = [ident]
    for d in range(4):
        blk = np.zeros((128, 512), np.float32)
        blk[:, :128 * d] = -3000.0
        blk[:, 128 * d:128 * d + 128] = tri
        blocks.append(blk)
    blocks.append(tri01)
    mask_np = np.concatenate(blocks, axis=1).astype(BF16)  # [128, 128+2048+128]
    # per-batch / per-head-group pieces computed once, shared across cores
    xTs = [np.ascontiguousarray(x[b].T).astype(BF16) for b in range(B)]
    per_g = []
    for g in range(4):
        cq = slice(0 * C + g * DQ, 0 * C + (g + 1) * DQ)
        ck = slice(1 * C + g * DQ, 1 * C + (g + 1) * DQ)
        cv = slice(2 * C + g * DQ, 2 * C + (g + 1) * DQ)
        bq = b_attn[cq]
        bk = b_attn[ck]
        per_g.append({
            "wq": np.ascontiguousarray(w_attn[:, cq]).astype(BF16),
            "wk": np.ascontiguousarray(w_attn[:, ck]).astype(BF16),
            "wv": np.ascontiguousarray(w_attn[:, cv]).astype(BF16),
            "wp": np.ascontiguousarray(w_proj[g * DQ:(g + 1) * DQ, :]).astype(BF16),
            "bqk": np.stack([bq.reshape(2, 128),
                             bk.reshape(2, 128)]).astype(np.float32),
            "bv": np.broadcast_to(b_attn[cv], (128, DQ)).copy().astype(np.float32),
            "mask": mask_np,
        })
    in_maps = []
    for core in range(NCORES):
        b, g = divmod(core, 4)
        in_maps.append({"xT": xTs[b], **per_g[g]})
    return in_maps


def _get_runner():
    """Compile once and keep a reusable sharded executable (repeated
    kernel() calls skip jit retracing and recompilation)."""
    if "runner" in _NC_CACHE:
        return _NC_CACHE["runner"]
    import jax
    import numpy as _np
    from jax.sharding import Mesh, NamedSharding, PartitionSpec
    from jax.experimental.shard_map import shard_map
    from concourse import bass2jax, mybir

    nc = _NC_CACHE.setdefault("nc", build_nc())
    bass2jax.install_neuronx_cc_hook()
    partition_name = nc.partition_id_tensor.name if nc.partition_id_tensor else None
    in_names, out_names, out_avals, zero_outs = [], [], [], []
    for alloc in nc.m.functions[0].allocations:
        if not isinstance(alloc, mybir.MemoryLocationSet):
            continue
        name = alloc.memorylocations[0].name
        if alloc.kind == "ExternalInput":
            if name != partition_name:
                in_names.append(name)
        elif alloc.kind == "ExternalOutput":
            shape = tuple(alloc.tensor_shape)
            dtype = mybir.dt.np(alloc.dtype)
            out_names.append(name)
            out_avals.append(jax.core.ShapedArray(shape, dtype))
            zero_outs.append(_np.zeros(shape, dtype))
    n_params = len(in_names)
    all_in_names = list(in_names) + list(out_names)
    if partition_name is not None:
        all_in_names.append(partition_name)

    def _body(*args):
        operands = list(args)
        if partition_name is not None:
            operands.append(bass2jax.partition_id_tensor())
        outs = bass2jax._bass_exec_p.bind(
            *operands,
            out_avals=tuple(out_avals),
            in_names=tuple(all_in_names),
            out_names=tuple(out_names),
            lowering_input_output_aliases=(),
            sim_require_finite=True,
            sim_require_nnan=True,
            nc=nc,
        )
        return tuple(outs)

    devices = jax.devices()[:NCORES]
    mesh = Mesh(np.asarray(devices), ("core",))
    in_specs = (PartitionSpec("core"),) * (n_params + len(out_names))
    out_specs = (PartitionSpec("core"),) * len(out_names)
    sharded = jax.jit(shard_map(_body, mesh=mesh, in_specs=in_specs,
                                out_specs=out_specs, check_rep=False),
                      keep_unused=True)
    sharding = NamedSharding(mesh, PartitionSpec("core"))
    concat_zeros = [np.zeros((NCORES * z.shape[0], *z.shape[1:]), z.dtype)
                    for z in zero_outs]
    dev_zero = [jax.device_put(a, sharding) for a in concat_zeros]
    runner = dict(sharded=sharded, in_names=in_names, sharding=sharding,
                  dev_zero=dev_zero, out_names=out_names)
    _NC_CACHE["runner"] = runner
    return runner


def kernel(x, w_attn, b_attn, w_proj, b_proj):
    import jax

    r = _get_runner()
    in_maps = make_in_maps(x, w_attn, b_attn, w_proj)
    concat_in = [np.concatenate([in_maps[c][name] for c in range(NCORES)], axis=0)
                 for name in r["in_names"]]
    dev_in = [jax.device_put(a, r["sharding"]) for a in concat_in]
    outs = r["sharded"](*dev_in, *r["dev_zero"])
    out_full = np.asarray(outs[0])  # [NCORES*T, C]

    b_proj = np.asarray(b_proj, dtype=np.float32)
    out = np.zeros((B, T, C), np.float32)
    for core in range(NCORES):
        b = core // 4
        out[b] += out_full[core * T:(core + 1) * T].astype(np.float32)
    out += b_proj[None, None, :]
    return out
